# revision 4
# baseline (speedup 1.0000x reference)
"""RPE (relative-position-bias) attention kernel for Trainium2, 8-core SPMD.

Full op (per reference):
  qkv = x @ w_qkv.T -> split q,k,v heads (H=12, hd=64), q *= hd**-0.5
  attn = q @ k.T ; attn[:, :, 1:, 1:] += rpb_table[rel_idx]  (per head)
  attn = softmax(attn, -1) ; out = (attn @ v) @ w_proj.T + b_proj

Sharding: data-parallel over batch. B=64 -> 8 batches per core. Weights
and bias-derived planes replicated to all cores. No collectives.

Per-core program (all matmuls bf16 operands, fp32 PSUM accumulation):
  - Inputs arrive bf16 and pre-transposed from the host: xT [768,1576],
    wqkvT [768,2304] (q columns pre-scaled by hd**-0.5), wpT [768,768].
  - qT,kT [768,1576] = w_chunk.T @ xT (transposed layout). v in natural
    layout [tokens, head, 65] with a ones column (softmax denominators
    fall out of the AV matmul for free).
  - The relative-position bias enters as exp(bias): probs = exp(s) *
    expb, expb planes host-precomputed bf16 in the transposed
    orientation [k_tok, q_tok] per head PAIR (two heads side by side,
    394 columns). exp runs on the scalar engine straight out of PSUM;
    the expb multiply runs on DVE in SBUF (bf16 fast path).
  - Heads are processed in pairs: score tiles [128,394] hold two heads.
  - Softmax normalization: denominators live in row 64 of the AV PSUM
    tile. Reciprocals run on DVE (keeps the scalar engine's activation
    table pinned to Exp -- the scalar-recip variant paid a 1.3us
    Exp<->Recip table reload per wave). The 4 reciprocal rows of a wave
    pack into one [1, 4*394] tile so a single gpsimd partition_broadcast
    per wave fans them out; the normalize multiplies run on DVE.
  - Engine balance: exp on scalar; qk PSUM->SBUF casts split between
    scalar (Copy shares the Exp table, no reload) and DVE; v casts on
    gpsimd; norm muls + expb muls + recips on DVE.
  - qk GEMMs for pair p+1 are emitted in two halves: column chunks 0,1
    between pair p's waves (needed by p+1 wave 0), chunks 2,3 after
    (needed only by p+1 wave 1). This keeps the PE fed across the wave
    boundary instead of bunching all 48 GEMMs at one point.
  - out = attn_outT.T @ wpT + b_proj  (fp32 output).
"""
import sys

sys.path.insert(0, '/opt/trn_rl_repo')

from contextlib import ExitStack

import numpy as np

import concourse.bass as bass
import concourse.bacc as bacc
import concourse.tile as tile
from concourse import mybir

# ---- problem dims (hardcoded per contract) ----
NCORES = 8
B_FULL = 64
B = B_FULL // NCORES     # 8 batches per core
N = 197                  # tokens (196 patches + CLS)
NP = 196
C = 768
H = 12
HD = 64
R = B * N                # 1576 rows per core
NPAIR = H // 2           # 6 head pairs
W2 = 2 * N               # 394 columns for a head pair

F32 = mybir.dt.float32
BF16 = mybir.dt.bfloat16
AF = mybir.ActivationFunctionType

import os
# CoreSim rejects reads of uninitialized PSUM; the exp deliberately reads a
# dead corner of the score tile (rows 69:128 of the chunk-1 columns, never
# consumed downstream). Sim runs memset it; hardware runs skip the cost.
SIM_SAFE = int(os.environ.get("KERNEL_SIM_SAFE", "0"))


def build_program():
    nc = bacc.Bacc("TRN2", target_bir_lowering=False, debug=False)

    x_d = nc.declare_dram_parameter("xT", [C, R], BF16, isOutput=False)
    wqkv_d = nc.declare_dram_parameter("w_qkvT", [C, 3 * C], BF16, isOutput=False)
    wp_d = nc.declare_dram_parameter("w_projT", [C, C], BF16, isOutput=False)
    bp_d = nc.declare_dram_parameter("b_proj", [C], F32, isOutput=False)
    # exp(bias) planes per head, transposed chunk-paired orientation:
    # expb [head, k_part 0:128, q 0:197 (k chunk 0) ++ q 0:197 (k chunk 1)]
    # (chunk 1 rows beyond k=196 are 1.0 and multiply unused garbage)
    expb_d = nc.declare_dram_parameter("expb", [H, 128, W2], BF16,
                                       isOutput=False)
    out_d = nc.declare_dram_parameter("out", [R, C], BF16, isOutput=True)

    with tile.TileContext(nc) as tc:
        with ExitStack() as ctx:
            _emit(ctx, tc, nc, x_d, wqkv_d, wp_d, bp_d, expb_d, out_d)
    nc.compile()
    return nc


def _emit(ctx, tc, nc, x_d, wqkv_d, wp_d, bp_d, expb_d, out_d):
    singles = ctx.enter_context(tc.tile_pool(name="singles", bufs=1))
    ps_pool = ctx.enter_context(tc.tile_pool(name="ps", bufs=4, space="PSUM"))
    av_ps = ctx.enter_context(tc.tile_pool(name="av_ps", bufs=4, space="PSUM"))
    probs_pool = ctx.enter_context(tc.tile_pool(name="probs", bufs=8))
    rec_pool = ctx.enter_context(tc.tile_pool(name="rec", bufs=2))
    out_pool = ctx.enter_context(tc.tile_pool(name="outp", bufs=6))

    KC = C // 128  # 6 contraction chunks

    # ---------------- load operands (already bf16 + transposed) ----------
    xT = []     # 6 x [128, R] bf16
    wqkvT = []  # 6 x [128, 2304] bf16
    wpT = []    # 6 x [128, 768] bf16
    for kc in range(KC):
        t = singles.tile([128, R], BF16, tag=f"xT{kc}", name=f"xT{kc}")
        nc.gpsimd.dma_start(out=t[:], in_=x_d[128 * kc:128 * (kc + 1), :])
        xT.append(t)
        w = singles.tile([128, 3 * C], BF16, tag=f"wqkvT{kc}", name=f"wqkvT{kc}")
        nc.sync.dma_start(out=w[:, 0:2 * C],
                          in_=wqkv_d[128 * kc:128 * (kc + 1), 0:2 * C])
        wqkvT.append(w)
    for kc in range(KC):
        nc.gpsimd.dma_start(out=wqkvT[kc][:, 2 * C:3 * C],
                            in_=wqkv_d[128 * kc:128 * (kc + 1), 2 * C:3 * C])
    for kc in range(KC):
        t = singles.tile([128, C], BF16, tag=f"wpT{kc}", name=f"wpT{kc}")
        nc.sync.dma_start(out=t[:], in_=wp_d[128 * kc:128 * (kc + 1), :])
        wpT.append(t)

    bproj_bc = singles.tile([128, C], F32, tag="bproj")
    nc.gpsimd.dma_start(out=bproj_bc[:],
                        in_=bass.AP(tensor=bp_d, offset=0, ap=[[0, 128], [1, C]]))

    expb = []  # [128, 394] bf16 per head (chunk-paired columns)
    for h in range(H):
        t0 = singles.tile([128, W2], BF16, tag=f"expb_{h}", name=f"expb_{h}")
        nc.sync.dma_start(out=t0[:], in_=expb_d[h, :, :])
        expb.append(t0)

    # ---------------- QKV ----------------
    NCHUNK = 4
    NW = R // NCHUNK  # 394 columns per psum tile

    # qk_sb[ft][ncol]: SEPARATE [128, 394] tiles per column chunk (2
    # batches each). Dependency tracking is tile-granular for matmul
    # operands, so per-chunk tiles let a wave's scores wait on exactly the
    # one copy they need instead of all four.
    qk_sb = [[None] * NCHUNK for _ in range(12)]

    def emit_qk(ft):
        # kc-outer, all four column tiles held across the contraction: the
        # first chunks compute while later weight DMA chunks still arrive
        # (only matters for the ramp pair ft 0/6)
        tiles = [ps_pool.tile([128, NW], F32, tag="ps", name=f"qkps{ncol}")
                 for ncol in range(NCHUNK)]
        for kc in range(KC):
            for ncol in range(NCHUNK):
                nc.tensor.matmul(
                    out=tiles[ncol][:],
                    lhsT=wqkvT[kc][:, 128 * ft:128 * (ft + 1)],
                    rhs=xT[kc][:, NW * ncol:NW * (ncol + 1)],
                    start=(kc == 0), stop=(kc == KC - 1))
        for ncol in range(NCHUNK):
            dst = singles.tile([128, NW], BF16, tag=f"qk{ft}_{ncol}",
                               name=f"qk{ft}_{ncol}")
            qk_sb[ft][ncol] = dst
            nc.vector.tensor_copy(out=dst[:], in_=tiles[ncol][:])

    def emit_qk_pair(fta, ftb, ncols):
        # q/k tile pair, GEMMs + copies interleaved per column chunk.
        # Copies alternate DVE / scalar-Copy (Copy shares the Exp act
        # table, so no table reload) to halve the per-chunk cast latency.
        for ncol in ncols:
            for i, ft in enumerate((fta, ftb)):
                ps = ps_pool.tile([128, NW], F32, tag="ps", name="qkps")
                for kc in range(KC):
                    nc.tensor.matmul(
                        out=ps[:],
                        lhsT=wqkvT[kc][:, 128 * ft:128 * (ft + 1)],
                        rhs=xT[kc][:, NW * ncol:NW * (ncol + 1)],
                        start=(kc == 0), stop=(kc == KC - 1))
                dst = singles.tile([128, NW], BF16, tag=f"qk{ft}_{ncol}",
                                   name=f"qk{ft}_{ncol}")
                qk_sb[ft][ncol] = dst
                if (ncol + i) % 2 == 0:
                    nc.vector.tensor_copy(out=dst[:], in_=ps[:])
                else:
                    nc.scalar.activation(out=dst[:], in_=ps[:], func=AF.Copy)

    # v_aug[b][c]: [128, 12, 65] bf16 (col 64 = ones)
    v_aug = [[None, None] for _ in range(B)]

    def emit_v(batches):
        for b in batches:
            for cchunk, (r0, nr) in enumerate(((N * b, 128), (N * b + 128, N - 128))):
                dst = singles.tile([128, H, HD + 1], BF16, tag=f"v{b}_{cchunk}",
                                   name=f"v{b}_{cchunk}")
                v_aug[b][cchunk] = dst
                nc.vector.memset(dst[:, :, HD:HD + 1], 1.0)
                for nh in range(2):
                    ps = ps_pool.tile([128, 384], F32, tag="ps")
                    for kc in range(KC):
                        nc.tensor.matmul(
                            out=ps[:nr, :],
                            lhsT=xT[kc][:, r0:r0 + nr],
                            rhs=wqkvT[kc][:, 2 * C + 384 * nh:2 * C + 384 * (nh + 1)],
                            start=(kc == 0), stop=(kc == KC - 1))
                    # PSUM->SBUF casts split DVE / scalar-Copy (gpsimd
                    # cannot read PSUM); both engines are idle at this
                    # point in the schedule
                    if (b + cchunk + nh) % 2 == 0:
                        nc.vector.tensor_copy(
                            out=dst[:nr, 6 * nh:6 * (nh + 1), 0:HD],
                            in_=ps[:nr, :].rearrange("p (h d) -> p h d", h=6))
                    else:
                        nc.scalar.activation(
                            out=dst[:nr, 6 * nh:6 * (nh + 1), 0:HD],
                            in_=ps[:nr, :].rearrange("p (h d) -> p h d", h=6),
                            func=AF.Copy)

    # deferred normalize closures (one per attention wave)
    pending_norm = []

    def flush_norm():
        while pending_norm:
            pending_norm.pop(0)()

    # attn output, transposed: 6 tiles [128, R] bf16 (pair p = heads 2p,2p+1)
    attn_outT = []
    for p in range(NPAIR):
        attn_outT.append(singles.tile([128, R], BF16, tag=f"aoT{p}",
                                      name=f"aoT{p}"))

    def emit_attention_pair(p, waves=(0, 1), mid_cb=None, end_cb=None):
        N1 = N - 128  # 69
        dst = attn_outT[p]
        for wave in waves:
            avs = []
            c0s = []

            def emit_scores(j):
                b = 4 * wave + j
                c0s.append(N * b)
                qt = qk_sb[p][b // 2]
                kt = qk_sb[6 + p][b // 2]
                o = N * (b % 2)
                qh = [qt[0:64, o:o + N], qt[64:128, o:o + N]]
                kh = [kt[0:64, o:o + N], kt[64:128, o:o + N]]
                phs = []
                for hh in range(2):
                    sth = ps_pool.tile([128, W2], F32, tag="ps",
                                       name=f"sth{hh}")
                    if SIM_SAFE:
                        nc.vector.memset(sth[64:128, N:W2], 0.0)
                    nc.tensor.matmul(out=sth[:, 0:N],
                                     lhsT=kh[hh][:, 0:128], rhs=qh[hh],
                                     start=True, stop=True)
                    nc.tensor.matmul(out=sth[0:N1, N:W2],
                                     lhsT=kh[hh][:, 128:N], rhs=qh[hh],
                                     start=True, stop=True)
                    ph = probs_pool.tile([128, W2], BF16, tag="probs")
                    nc.scalar.activation(out=ph[:], in_=sth[:], func=AF.Exp)
                    # expb multiply is all-SBUF bf16: legal on gpsimd, so
                    # split the 16 per pair between DVE and gpsimd
                    eng = nc.vector if hh == 0 else nc.gpsimd
                    eng.tensor_mul(out=ph[:], in0=ph[:],
                                   in1=expb[2 * p + hh][:])
                    phs.append(ph)
                return phs

            def emit_av(j, phs):
                b = 4 * wave + j
                av = av_ps.tile([HD + 1, W2], F32, tag="av")
                avs.append(av)
                for hh in range(2):
                    h = 2 * p + hh
                    nc.tensor.matmul(out=av[:, N * hh:N * hh + N],
                                     lhsT=v_aug[b][0][:, h, :],
                                     rhs=phs[hh][:, 0:N],
                                     start=True, stop=False)
                    nc.tensor.matmul(out=av[:, N * hh:N * hh + N],
                                     lhsT=v_aug[b][1][0:N1, h, :],
                                     rhs=phs[hh][0:N1, N:W2],
                                     start=False, stop=True)

            pending = [emit_scores(0), emit_scores(1)]
            # previous wave's normalize runs here: its DVE recips/muls land
            # behind this wave's expb multiplies, off the score->AV path
            flush_norm()
            for j in range(4):
                emit_av(j, pending[j])
                if j + 2 < 4:
                    pending.append(emit_scores(j + 2))

            def norm(avs=avs, c0s=list(c0s), dst=dst):
                # pack the wave's 4 reciprocal rows into one tile so a
                # single gpsimd partition_broadcast fans them all out
                rec4 = rec_pool.tile([1, 4 * W2], F32, tag="rec4")
                for j in range(4):
                    nc.vector.reciprocal(out=rec4[0:1, W2 * j:W2 * (j + 1)],
                                         in_=avs[j][HD:HD + 1, :])
                rec_sb = rec_pool.tile([128, 4 * W2], F32, tag="rec_sb")
                nc.gpsimd.partition_broadcast(rec_sb[:], rec4[0:1, :])
                for j in range(4):
                    o = W2 * j
                    nc.vector.tensor_mul(out=dst[0:64, c0s[j]:c0s[j] + N],
                                         in0=avs[j][0:HD, 0:N],
                                         in1=rec_sb[0:64, o:o + N])
                    nc.vector.tensor_mul(out=dst[64:128, c0s[j]:c0s[j] + N],
                                         in0=avs[j][0:HD, N:W2],
                                         in1=rec_sb[64:128, o + N:o + W2])
            pending_norm.append(norm)
            if wave == waves[0] and mid_cb is not None:
                mid_cb()
        if end_cb is not None:
            end_cb()

    # ---------------- proj ----------------
    NRC = (R + 127) // 128  # 13 row chunks

    def emit_proj(rcs):
        for rc in rcs:
            r0 = 128 * rc
            nr = min(128, R - r0)
            for nh in range(2):
                ps = ps_pool.tile([128, 384], F32, tag="ps")
                for kc in range(KC):
                    nc.tensor.matmul(
                        out=ps[:nr, :],
                        lhsT=attn_outT[kc][:, r0:r0 + nr],
                        rhs=wpT[kc][:, 384 * nh:384 * (nh + 1)],
                        start=(kc == 0), stop=(kc == KC - 1))
                ot = out_pool.tile([128, 384], BF16, tag="out")
                nc.vector.tensor_add(out=ot[:nr, :], in0=ps[:nr, :],
                                     in1=bproj_bc[:nr, 384 * nh:384 * (nh + 1)])
                # alternate store queues: descriptor issue (~0.6us each)
                # parallelizes across the sync and (by now idle) gpsimd DGE
                eng = nc.sync if (rc + nh) % 2 == 0 else nc.gpsimd
                eng.dma_start(
                    out=out_d[r0:r0 + nr, 384 * nh:384 * (nh + 1)],
                    in_=ot[:nr, :])

    # ---------------- emission schedule ----------------
    # ramp: q/k for pair 0, v for the first wave's batches
    emit_qk(0)
    emit_qk(6)
    emit_v(range(0, 4))

    def _mk_mid(pn, first):
        # between pair p's waves: (pair 0 only) v for wave-1 batches, then
        # the next pair's qk column chunks 0,1 (needed by its wave 0)
        def cb():
            if first:
                emit_v(range(4, 8))
            emit_qk_pair(pn, 6 + pn, ncols=(0, 1))
        return cb

    def _mk_end(pn):
        # after pair p's wave 1: next pair's qk chunks 2,3 (needed only by
        # its wave 1 -- plenty of slack)
        return lambda: emit_qk_pair(pn, 6 + pn, ncols=(2, 3))

    for p in range(NPAIR - 1):
        emit_attention_pair(p, mid_cb=_mk_mid(p + 1, p == 0),
                            end_cb=_mk_end(p + 1))

    # last pair: wave 0 (batches 0-3), then the proj row-chunks those
    # batches complete, then wave 1, then the rest -- shrinks the tail
    emit_attention_pair(NPAIR - 1, waves=(0,))
    flush_norm()
    emit_proj(range(0, 6))
    emit_attention_pair(NPAIR - 1, waves=(1,))
    flush_norm()
    emit_proj(range(6, NRC))


_NC_CACHE = {}


def _get_nc():
    if "nc" not in _NC_CACHE:
        _NC_CACHE["nc"] = build_program()
    return _NC_CACHE["nc"]


def prep_aux(rpb_table, rel_idx):
    """Host-side prep: gather the bias from the two small aux inputs, lay it
    out per head PAIR in the kernel's transposed plane orientation
    [k_tok, q_tok*2] with zeroed CLS row/col, and exponentiate (bf16)."""
    import ml_dtypes
    bT = rpb_table[rel_idx.reshape(-1)].reshape(NP, NP, H)  # [q_idx, k_idx, h]
    bT = np.ascontiguousarray(bT.transpose(1, 0, 2))        # [k_idx, q_idx, h]
    bias0 = np.zeros((128, N, H), dtype=np.float32)
    bias0[1:128, 1:NP + 1, :] = bT[0:127]
    bias1 = np.zeros((128, N, H), dtype=np.float32)
    bias1[0:NP - 127, 1:NP + 1, :] = bT[127:NP]
    expb = np.zeros((H, 128, W2), dtype=np.float32)
    for h in range(H):
        expb[h, :, 0:N] = np.exp(bias0[:, :, h])
        expb[h, :, N:W2] = np.exp(bias1[:, :, h])
    return expb.astype(ml_dtypes.bfloat16)


def prep_weights(w_qkv, w_proj):
    """Host-side prep: transpose, fold the q scale into w_qkv, cast bf16."""
    import ml_dtypes
    wqkvT = np.array(w_qkv, dtype=np.float32).T.copy()
    wqkvT[:, 0:C] *= HD ** -0.5
    wpT = np.ascontiguousarray(np.asarray(w_proj, dtype=np.float32).T)
    return (wqkvT.astype(ml_dtypes.bfloat16), wpT.astype(ml_dtypes.bfloat16))


def make_in_maps(x, w_qkv, w_proj, b_proj, rpb_table, rel_idx):
    """Build the 8 per-core input maps (host prep: shard, transpose, bf16)."""
    import ml_dtypes
    x = np.asarray(x, dtype=np.float32)
    expb = prep_aux(
        np.asarray(rpb_table, dtype=np.float32), np.asarray(rel_idx).astype(np.int64))
    wqkvT, wpT = prep_weights(w_qkv, w_proj)
    bp = np.ascontiguousarray(np.asarray(b_proj, dtype=np.float32))
    xbf = x.astype(ml_dtypes.bfloat16)
    in_maps = []
    for c in range(NCORES):
        xT = np.ascontiguousarray(xbf[c * B:(c + 1) * B].reshape(R, C).T)
        in_maps.append({
            "xT": xT,
            "w_qkvT": wqkvT,
            "w_projT": wpT,
            "b_proj": bp,
            "expb": expb,
        })
    return in_maps


def kernel(x, w_qkv, w_proj, b_proj, rpb_table, rel_idx):
    from concourse.bass_utils import run_bass_kernel_spmd

    nc = _get_nc()
    in_maps = make_in_maps(x, w_qkv, w_proj, b_proj, rpb_table, rel_idx)
    res = run_bass_kernel_spmd(nc, in_maps, list(range(NCORES)))
    out = np.concatenate(
        [np.asarray(r["out"], dtype=np.float32).reshape(B, N, C)
         for r in res.results], axis=0)
    return out


# revision 20
# speedup vs baseline: 1.2196x; 1.2196x over previous
"""RPE (relative-position-bias) attention kernel for Trainium2, 8-core SPMD.

Full op (per reference):
  qkv = x @ w_qkv.T -> split q,k,v heads (H=12, hd=64), q *= hd**-0.5
  attn = q @ k.T ; attn[:, :, 1:, 1:] += rpb_table[rel_idx]  (per head)
  attn = softmax(attn, -1) ; out = (attn @ v) @ w_proj.T + b_proj

Sharding: data-parallel over batch. B=64 -> 8 batches per core. Weights
and bias-derived planes replicated to all cores. No collectives.

Per-core program (all matmuls bf16 operands, fp32 PSUM accumulation):
  - Inputs arrive bf16 and pre-transposed from the host: xT [768,1576],
    wqkvT [768,2304] (q columns pre-scaled by hd**-0.5), wpT [768,768].
  - qT,kT [768,1576] = w_chunk.T @ xT (transposed layout). v in natural
    layout [tokens, head, 65] with a ones column (softmax denominators
    fall out of the AV matmul for free).
  - The relative-position bias enters as exp(bias): probs = exp(s) *
    expb, expb planes host-precomputed bf16 in the transposed
    orientation [k_tok, q_tok] per head PAIR (two heads side by side,
    394 columns). exp runs on the scalar engine straight out of PSUM;
    the expb multiply runs on DVE in SBUF (bf16 fast path).
  - Heads are processed in pairs: score tiles [128,394] hold two heads.
  - Softmax normalization: denominators live in row 64 of the AV PSUM
    tile. Reciprocals run on DVE (keeps the scalar engine's activation
    table pinned to Exp -- the scalar-recip variant paid a 1.3us
    Exp<->Recip table reload per wave). The 4 reciprocal rows of a wave
    pack into one [1, 4*394] tile so a single gpsimd partition_broadcast
    per wave fans them out; the normalize multiplies run on DVE.
  - Engine balance: exp on scalar; qk PSUM->SBUF casts split between
    scalar (Copy shares the Exp table, no reload) and DVE; v casts on
    gpsimd; norm muls + expb muls + recips on DVE.
  - qk GEMMs for pair p+1 are emitted in two halves: column chunks 0,1
    between pair p's waves (needed by p+1 wave 0), chunks 2,3 after
    (needed only by p+1 wave 1). This keeps the PE fed across the wave
    boundary instead of bunching all 48 GEMMs at one point.
  - out = attn_outT.T @ wpT + b_proj  (fp32 output).
"""
import sys

sys.path.insert(0, '/opt/trn_rl_repo')

from contextlib import ExitStack

import numpy as np

import concourse.bass as bass
import concourse.bacc as bacc
import concourse.tile as tile
from concourse import mybir

# ---- problem dims (hardcoded per contract) ----
NCORES = 8
B_FULL = 64
B = B_FULL // NCORES     # 8 batches per core
N = 197                  # tokens (196 patches + CLS)
NP = 196
C = 768
H = 12
HD = 64
R = B * N                # 1576 rows per core
NPAIR = H // 2           # 6 head pairs
W2 = 2 * N               # 394 columns for a head pair

F32 = mybir.dt.float32
BF16 = mybir.dt.bfloat16
AF = mybir.ActivationFunctionType

import os
# CoreSim rejects reads of uninitialized PSUM; the exp deliberately reads a
# dead corner of the score tile (rows 69:128 of the chunk-1 columns, never
# consumed downstream). Sim runs memset it; hardware runs skip the cost.
SIM_SAFE = int(os.environ.get("KERNEL_SIM_SAFE", "0"))


def build_program():
    nc = bacc.Bacc("TRN2", target_bir_lowering=False, debug=False)

    x_d = nc.declare_dram_parameter("xT", [C, R], BF16, isOutput=False)
    wqkv_d = nc.declare_dram_parameter("w_qkvT", [C, 3 * C], BF16, isOutput=False)
    wp_d = nc.declare_dram_parameter("w_projT", [C, C], BF16, isOutput=False)
    bp_d = nc.declare_dram_parameter("b_proj", [C], F32, isOutput=False)
    # exp(bias) planes per head, transposed chunk-paired orientation:
    # expb [k_part 0:128, head, q 0:197 (k chunk 0) ++ q 0:197 (k chunk 1)]
    # (chunk 1 rows beyond k=196 are 1.0 and multiply unused garbage);
    # partition-major host layout so it loads as ONE contiguous DMA
    expb_d = nc.declare_dram_parameter("expb", [128, H * W2], BF16,
                                       isOutput=False)
    out_d = nc.declare_dram_parameter("out", [R, C], BF16, isOutput=True)

    with tile.TileContext(nc) as tc:
        with ExitStack() as ctx:
            _emit(ctx, tc, nc, x_d, wqkv_d, wp_d, bp_d, expb_d, out_d)
    nc.compile()
    return nc


def _emit(ctx, tc, nc, x_d, wqkv_d, wp_d, bp_d, expb_d, out_d):
    singles = ctx.enter_context(tc.tile_pool(name="singles", bufs=1))
    ps_pool = ctx.enter_context(tc.tile_pool(name="ps", bufs=4, space="PSUM"))
    av_ps = ctx.enter_context(tc.tile_pool(name="av_ps", bufs=4, space="PSUM"))
    probs_pool = ctx.enter_context(tc.tile_pool(name="probs", bufs=8))
    rec_pool = ctx.enter_context(tc.tile_pool(name="rec", bufs=2))
    out_pool = ctx.enter_context(tc.tile_pool(name="outp", bufs=6))

    KC = C // 128  # 6 contraction chunks

    # ---------------- load operands (already bf16 + transposed) ----------
    xT = []     # 6 x [128, R] bf16
    wqkvT = []  # 6 x [128, 1536] bf16 (q,k columns; v loads separately)
    for kc in range(KC):
        t = singles.tile([128, R], BF16, tag=f"xT{kc}", name=f"xT{kc}")
        nc.gpsimd.dma_start(out=t[:], in_=x_d[128 * kc:128 * (kc + 1), :])
        xT.append(t)
        w = singles.tile([128, 2 * C], BF16, tag=f"wqkvT{kc}", name=f"wqkvT{kc}")
        nc.sync.dma_start(out=w[:],
                          in_=wqkv_d[128 * kc:128 * (kc + 1), 0:2 * C])
        wqkvT.append(w)
    # v columns of w_qkv: one strided DMA filling all six chunk tiles'
    # v-column ranges (6 descriptors/partition instead of 6 DMA issues)
    vcols = singles.tile([128, KC, C], BF16, tag="wqkv_v", name="wqkv_v")
    nc.gpsimd.dma_start(
        out=vcols[:],
        in_=bass.AP(tensor=wqkv_d, offset=2 * C,
                    ap=[[3 * C, 128], [128 * 3 * C, KC], [1, C]]))
    # wp: one strided DMA for all 6 contraction chunks
    wp_all = singles.tile([128, KC, C], BF16, tag="wp_all", name="wp_all")
    nc.sync.dma_start(
        out=wp_all[:],
        in_=bass.AP(tensor=wp_d, offset=0,
                    ap=[[C, 128], [128 * C, KC], [1, C]]))

    bproj_bc = singles.tile([128, C], F32, tag="bproj")
    nc.gpsimd.dma_start(out=bproj_bc[:],
                        in_=bass.AP(tensor=bp_d, offset=0, ap=[[0, 128], [1, C]]))

    expb_all = singles.tile([128, H, W2], BF16, tag="expb", name="expb")
    nc.sync.dma_start(out=expb_all[:], in_=expb_d[:, :])
    expb = [expb_all[:, h, :] for h in range(H)]  # [128, 394] per head

    # ---------------- QKV ----------------
    NCHUNK = 4
    NW = R // NCHUNK  # 394 columns per psum tile

    # qk_sb[ft][ncol]: SEPARATE [128, 394] tiles per column chunk (2
    # batches each). Dependency tracking is tile-granular for matmul
    # operands, so per-chunk tiles let a wave's scores wait on exactly the
    # one copy they need instead of all four.
    qk_sb = [[None] * NCHUNK for _ in range(12)]

    def emit_qk(ft):
        # kc-outer, all four column tiles held across the contraction: the
        # first chunks compute while later weight DMA chunks still arrive
        # (only matters for the ramp pair ft 0/6)
        tiles = [ps_pool.tile([128, NW], F32, tag="ps", name=f"qkps{ncol}")
                 for ncol in range(NCHUNK)]
        for kc in range(KC):
            for ncol in range(NCHUNK):
                nc.tensor.matmul(
                    out=tiles[ncol][:],
                    lhsT=wqkvT[kc][:, 128 * ft:128 * (ft + 1)],
                    rhs=xT[kc][:, NW * ncol:NW * (ncol + 1)],
                    start=(kc == 0), stop=(kc == KC - 1))
        for ncol in range(NCHUNK):
            dst = singles.tile([128, NW], BF16, tag=f"qk{ft}_{ncol}",
                               name=f"qk{ft}_{ncol}")
            qk_sb[ft][ncol] = dst
            nc.vector.tensor_copy(out=dst[:], in_=tiles[ncol][:])

    def emit_qk_pair(fta, ftb, ncols):
        # q/k tile pair, GEMMs + copies interleaved per column chunk.
        # Copies run as scalar-Copy (Copy shares the Exp act table, so no
        # table reload) -- DVE carries the recip/norm/expb work instead.
        for ncol in ncols:
            for i, ft in enumerate((fta, ftb)):
                ps = ps_pool.tile([128, NW], F32, tag="ps", name="qkps")
                for kc in range(KC):
                    nc.tensor.matmul(
                        out=ps[:],
                        lhsT=wqkvT[kc][:, 128 * ft:128 * (ft + 1)],
                        rhs=xT[kc][:, NW * ncol:NW * (ncol + 1)],
                        start=(kc == 0), stop=(kc == KC - 1))
                dst = singles.tile([128, NW], BF16, tag=f"qk{ft}_{ncol}",
                                   name=f"qk{ft}_{ncol}")
                qk_sb[ft][ncol] = dst
                if (ncol + i) % 2 == 0:
                    nc.scalar.activation(out=dst[:], in_=ps[:], func=AF.Copy)
                else:
                    nc.vector.tensor_copy(out=dst[:], in_=ps[:])

    # v_aug[b][c]: [128, 12, 65] bf16 (col 64 = ones)
    v_aug = [[None, None] for _ in range(B)]

    def emit_v(batches):
        for b in batches:
            for cchunk, (r0, nr) in enumerate(((N * b, 128), (N * b + 128, N - 128))):
                dst = singles.tile([128, H, HD + 1], BF16, tag=f"v{b}_{cchunk}",
                                   name=f"v{b}_{cchunk}")
                v_aug[b][cchunk] = dst
                nc.vector.memset(dst[:, :, HD:HD + 1], 1.0)
                for nh in range(2):
                    ps = ps_pool.tile([128, 384], F32, tag="ps")
                    for kc in range(KC):
                        nc.tensor.matmul(
                            out=ps[:nr, :],
                            lhsT=xT[kc][:, r0:r0 + nr],
                            rhs=vcols[:, kc, 384 * nh:384 * (nh + 1)],
                            start=(kc == 0), stop=(kc == KC - 1))
                    # PSUM->SBUF casts split DVE / scalar-Copy (gpsimd
                    # cannot read PSUM); both engines are idle at this
                    # point in the schedule
                    if (b + cchunk + nh) % 2 == 0:
                        nc.vector.tensor_copy(
                            out=dst[:nr, 6 * nh:6 * (nh + 1), 0:HD],
                            in_=ps[:nr, :].rearrange("p (h d) -> p h d", h=6))
                    else:
                        nc.scalar.activation(
                            out=dst[:nr, 6 * nh:6 * (nh + 1), 0:HD],
                            in_=ps[:nr, :].rearrange("p (h d) -> p h d", h=6),
                            func=AF.Copy)

    # deferred normalize closures (one per attention wave)
    pending_norm = []

    def flush_norm():
        while pending_norm:
            pending_norm.pop(0)()

    # attn output, transposed: 6 tiles [128, R] bf16 (pair p = heads 2p,2p+1)
    attn_outT = []
    for p in range(NPAIR):
        attn_outT.append(singles.tile([128, R], BF16, tag=f"aoT{p}",
                                      name=f"aoT{p}"))

    def emit_attention_pair(p, waves=(0, 1), mid_cb=None, end_cb=None):
        N1 = N - 128  # 69
        dst = attn_outT[p]
        for wave in waves:
            avs = []
            c0s = []

            def emit_scores(j):
                b = 4 * wave + j
                c0s.append(N * b)
                qt = qk_sb[p][b // 2]
                kt = qk_sb[6 + p][b // 2]
                o = N * (b % 2)
                qh = [qt[0:64, o:o + N], qt[64:128, o:o + N]]
                kh = [kt[0:64, o:o + N], kt[64:128, o:o + N]]
                phs = []
                for hh in range(2):
                    sth = ps_pool.tile([128, W2], F32, tag="ps",
                                       name=f"sth{hh}")
                    if SIM_SAFE:
                        nc.vector.memset(sth[64:128, N:W2], 0.0)
                    nc.tensor.matmul(out=sth[:, 0:N],
                                     lhsT=kh[hh][:, 0:128], rhs=qh[hh],
                                     start=True, stop=True)
                    nc.tensor.matmul(out=sth[0:N1, N:W2],
                                     lhsT=kh[hh][:, 128:N], rhs=qh[hh],
                                     start=True, stop=True)
                    ph = probs_pool.tile([128, W2], BF16, tag="probs")
                    nc.scalar.activation(out=ph[:], in_=sth[:], func=AF.Exp)
                    # expb multiply is all-SBUF bf16 (fast DVE path); the
                    # gpsimd Pool engine takes 2 of the 8 per wave to shave
                    # DVE occupancy (Pool is ~4x slower per op but idle)
                    eng = nc.gpsimd if (hh == 1 and b % 2 == 1) else nc.vector
                    eng.tensor_mul(out=ph[:], in0=ph[:],
                                   in1=expb[2 * p + hh])
                    phs.append(ph)
                return phs

            def emit_av(j, phs):
                b = 4 * wave + j
                av = av_ps.tile([HD + 1, W2], F32, tag="av")
                avs.append(av)
                for hh in range(2):
                    h = 2 * p + hh
                    nc.tensor.matmul(out=av[:, N * hh:N * hh + N],
                                     lhsT=v_aug[b][0][:, h, :],
                                     rhs=phs[hh][:, 0:N],
                                     start=True, stop=False)
                    nc.tensor.matmul(out=av[:, N * hh:N * hh + N],
                                     lhsT=v_aug[b][1][0:N1, h, :],
                                     rhs=phs[hh][0:N1, N:W2],
                                     start=False, stop=True)

            pending = [emit_scores(0), emit_scores(1)]
            # previous wave's normalize runs here: its DVE recips/muls land
            # behind this wave's expb multiplies, off the score->AV path
            flush_norm()
            for j in range(4):
                emit_av(j, pending[j])
                if j + 2 < 4:
                    pending.append(emit_scores(j + 2))

            def norm(avs=avs, c0s=list(c0s), dst=dst):
                # the reciprocal runs on DVE as a single packed
                # reciprocal_approx_fast (~18-bit; the exact InstReciprocal
                # measures 2.6us per row, and the scalar-engine recip costs
                # a 1.3us Exp<->Recip act-table reload per wave). The custom
                # DVE op reads garbage from PSUM, so the 4 denominator rows
                # are first copied to SBUF (split scalar/DVE).
                d4 = rec_pool.tile([1, 4 * W2], F32, tag="d4")
                for j in range(4):
                    if j % 2 == 0:
                        nc.scalar.activation(
                            out=d4[0:1, W2 * j:W2 * (j + 1)],
                            in_=avs[j][HD:HD + 1, :], func=AF.Copy)
                    else:
                        nc.vector.tensor_copy(
                            out=d4[0:1, W2 * j:W2 * (j + 1)],
                            in_=avs[j][HD:HD + 1, :])
                rec4 = rec_pool.tile([1, 4 * W2], F32, tag="rec4")
                nc.vector.reciprocal_approx_fast(out=rec4[0:1, :],
                                                 in_=d4[0:1, :])
                rec_sb = rec_pool.tile([128, 4 * W2], F32, tag="rec_sb")
                nc.gpsimd.partition_broadcast(rec_sb[:], rec4[0:1, :])
                for j in range(4):
                    o = W2 * j
                    nc.vector.tensor_mul(out=dst[0:64, c0s[j]:c0s[j] + N],
                                         in0=avs[j][0:HD, 0:N],
                                         in1=rec_sb[0:64, o:o + N])
                    nc.vector.tensor_mul(out=dst[64:128, c0s[j]:c0s[j] + N],
                                         in0=avs[j][0:HD, N:W2],
                                         in1=rec_sb[64:128, o + N:o + W2])
            pending_norm.append(norm)
            if wave == waves[0] and mid_cb is not None:
                mid_cb()
        if end_cb is not None:
            end_cb()

    # ---------------- proj ----------------
    NRC = (R + 127) // 128  # 13 row chunks

    def emit_proj(rcs):
        for rc in rcs:
            r0 = 128 * rc
            nr = min(128, R - r0)
            ot = out_pool.tile([128, C], BF16, tag="out")
            for nh in range(2):
                ps = ps_pool.tile([128, 384], F32, tag="ps")
                for kc in range(KC):
                    nc.tensor.matmul(
                        out=ps[:nr, :],
                        lhsT=attn_outT[kc][:, r0:r0 + nr],
                        rhs=wp_all[:, kc, 384 * nh:384 * (nh + 1)],
                        start=(kc == 0), stop=(kc == KC - 1))
                nc.vector.tensor_add(out=ot[:nr, 384 * nh:384 * (nh + 1)],
                                     in0=ps[:nr, :],
                                     in1=bproj_bc[:nr, 384 * nh:384 * (nh + 1)])
            # one [*, 768] store per row chunk; alternate store queues so
            # descriptor issue parallelizes across the sync and gpsimd DGE
            eng = nc.sync if rc % 2 == 0 else nc.gpsimd
            eng.dma_start(out=out_d[r0:r0 + nr, :], in_=ot[:nr, :])

    # ---------------- emission schedule ----------------
    # ramp: q/k for pair 0, v for the first wave's batches
    emit_qk(0)
    emit_qk(6)
    emit_v(range(0, 4))

    def _mk_mid(pn, first):
        # between pair p's waves: (pair 0 only) v for wave-1 batches, then
        # the next pair's qk column chunks 0,1 (needed by its wave 0)
        def cb():
            if first:
                emit_v(range(4, 8))
            emit_qk_pair(pn, 6 + pn, ncols=(0, 1))
        return cb

    def _mk_end(pn):
        # after pair p's wave 1: next pair's qk chunks 2,3 (needed only by
        # its wave 1 -- plenty of slack)
        return lambda: emit_qk_pair(pn, 6 + pn, ncols=(2, 3))

    for p in range(NPAIR - 1):
        emit_attention_pair(p, mid_cb=_mk_mid(p + 1, p == 0),
                            end_cb=_mk_end(p + 1))

    # last pair: wave 0 (batches 0-3), then the proj row-chunks those
    # batches complete, then wave 1, then the rest -- shrinks the tail
    emit_attention_pair(NPAIR - 1, waves=(0,))
    flush_norm()
    emit_proj(range(0, 6))
    emit_attention_pair(NPAIR - 1, waves=(1,))
    flush_norm()
    emit_proj(range(6, NRC))


_NC_CACHE = {}


def _get_nc():
    if "nc" not in _NC_CACHE:
        _NC_CACHE["nc"] = build_program()
    return _NC_CACHE["nc"]


def prep_aux(rpb_table, rel_idx):
    """Host-side prep: gather the bias from the two small aux inputs, lay it
    out per head PAIR in the kernel's transposed plane orientation
    [k_tok, q_tok*2] with zeroed CLS row/col, and exponentiate (bf16)."""
    import ml_dtypes
    bT = rpb_table[rel_idx.reshape(-1)].reshape(NP, NP, H)  # [q_idx, k_idx, h]
    bT = np.ascontiguousarray(bT.transpose(1, 0, 2))        # [k_idx, q_idx, h]
    bias0 = np.zeros((128, N, H), dtype=np.float32)
    bias0[1:128, 1:NP + 1, :] = bT[0:127]
    bias1 = np.zeros((128, N, H), dtype=np.float32)
    bias1[0:NP - 127, 1:NP + 1, :] = bT[127:NP]
    # partition-major [128, H, W2] so the device loads it as one DMA
    expb = np.zeros((128, H, W2), dtype=np.float32)
    for h in range(H):
        expb[:, h, 0:N] = np.exp(bias0[:, :, h])
        expb[:, h, N:W2] = np.exp(bias1[:, :, h])
    return expb.reshape(128, H * W2).astype(ml_dtypes.bfloat16)


def prep_weights(w_qkv, w_proj):
    """Host-side prep: transpose, fold the q scale into w_qkv, cast bf16."""
    import ml_dtypes
    wqkvT = np.array(w_qkv, dtype=np.float32).T.copy()
    wqkvT[:, 0:C] *= HD ** -0.5
    wpT = np.ascontiguousarray(np.asarray(w_proj, dtype=np.float32).T)
    return (wqkvT.astype(ml_dtypes.bfloat16), wpT.astype(ml_dtypes.bfloat16))


def make_in_maps(x, w_qkv, w_proj, b_proj, rpb_table, rel_idx):
    """Build the 8 per-core input maps (host prep: shard, transpose, bf16)."""
    import ml_dtypes
    x = np.asarray(x, dtype=np.float32)
    expb = prep_aux(
        np.asarray(rpb_table, dtype=np.float32), np.asarray(rel_idx).astype(np.int64))
    wqkvT, wpT = prep_weights(w_qkv, w_proj)
    bp = np.ascontiguousarray(np.asarray(b_proj, dtype=np.float32))
    xbf = x.astype(ml_dtypes.bfloat16)
    in_maps = []
    for c in range(NCORES):
        xT = np.ascontiguousarray(xbf[c * B:(c + 1) * B].reshape(R, C).T)
        in_maps.append({
            "xT": xT,
            "w_qkvT": wqkvT,
            "w_projT": wpT,
            "b_proj": bp,
            "expb": expb,
        })
    return in_maps


def kernel(x, w_qkv, w_proj, b_proj, rpb_table, rel_idx):
    from concourse.bass_utils import run_bass_kernel_spmd

    nc = _get_nc()
    in_maps = make_in_maps(x, w_qkv, w_proj, b_proj, rpb_table, rel_idx)
    res = run_bass_kernel_spmd(nc, in_maps, list(range(NCORES)))
    out = np.concatenate(
        [np.asarray(r["out"], dtype=np.float32).reshape(B, N, C)
         for r in res.results], axis=0)
    return out


# revision 25
# speedup vs baseline: 1.6189x; 1.3274x over previous
"""RPE (relative-position-bias) attention kernel for Trainium2, 8-core SPMD.

Full op (per reference):
  qkv = x @ w_qkv.T -> split q,k,v heads (H=12, hd=64), q *= hd**-0.5
  attn = q @ k.T ; attn[:, :, 1:, 1:] += rpb_table[rel_idx]  (per head)
  attn = softmax(attn, -1) ; out = (attn @ v) @ w_proj.T + b_proj

Sharding: data-parallel over batch. B=64 -> 8 batches per core. Weights
and bias-derived planes replicated to all cores. No collectives.

Per-core program (all matmuls bf16 operands, fp32 PSUM accumulation):
  - Inputs arrive bf16 and pre-transposed from the host: xT [768,1576],
    wqkvT [768,2304] (q columns pre-scaled by hd**-0.5), wpT [768,768].
  - qT,kT [768,1576] = w_chunk.T @ xT (transposed layout). v in natural
    layout [tokens, head, 65] with a ones column (softmax denominators
    fall out of the AV matmul for free).
  - The relative-position bias enters as exp(bias): probs = exp(s) *
    expb, expb planes host-precomputed bf16 in the transposed
    orientation [k_tok, q_tok] per head PAIR (two heads side by side,
    394 columns). exp runs on the scalar engine straight out of PSUM;
    the expb multiply runs on DVE in SBUF (bf16 fast path).
  - Heads are processed in pairs: score tiles [128,394] hold two heads.
  - Softmax normalization: denominators live in row 64 of the AV PSUM
    tile. Reciprocals run on DVE (keeps the scalar engine's activation
    table pinned to Exp -- the scalar-recip variant paid a 1.3us
    Exp<->Recip table reload per wave). The 4 reciprocal rows of a wave
    pack into one [1, 4*394] tile so a single gpsimd partition_broadcast
    per wave fans them out; the normalize multiplies run on DVE.
  - Engine balance: exp on scalar; qk PSUM->SBUF casts split between
    scalar (Copy shares the Exp table, no reload) and DVE; v casts on
    gpsimd; norm muls + expb muls + recips on DVE.
  - qk GEMMs for pair p+1 are emitted in two halves: column chunks 0,1
    between pair p's waves (needed by p+1 wave 0), chunks 2,3 after
    (needed only by p+1 wave 1). This keeps the PE fed across the wave
    boundary instead of bunching all 48 GEMMs at one point.
  - out = attn_outT.T @ wpT + b_proj  (fp32 output).
"""
import sys

sys.path.insert(0, '/opt/trn_rl_repo')

from contextlib import ExitStack

import numpy as np

import concourse.bass as bass
import concourse.bacc as bacc
import concourse.tile as tile
from concourse import mybir

# ---- problem dims (hardcoded per contract) ----
NCORES = 8
B_FULL = 64
B = B_FULL // NCORES     # 8 batches per core
N = 197                  # tokens (196 patches + CLS)
NP = 196
C = 768
H = 12
HD = 64
R = B * N                # 1576 rows per core
NPAIR = H // 2           # 6 head pairs
W2 = 2 * N               # 394 columns for a head pair

F32 = mybir.dt.float32
BF16 = mybir.dt.bfloat16
AF = mybir.ActivationFunctionType

import os
# CoreSim rejects reads of uninitialized PSUM; the exp deliberately reads a
# dead corner of the score tile (rows 69:128 of the chunk-1 columns, never
# consumed downstream). Sim runs memset it; hardware runs skip the cost.
SIM_SAFE = int(os.environ.get("KERNEL_SIM_SAFE", "0"))


def build_program():
    nc = bacc.Bacc("TRN2", target_bir_lowering=False, debug=False)

    x_d = nc.declare_dram_parameter("xT", [C, R], BF16, isOutput=False)
    wqkv_d = nc.declare_dram_parameter("w_qkvT", [C, 3 * C], BF16, isOutput=False)
    wp_d = nc.declare_dram_parameter("w_projT", [C, C], BF16, isOutput=False)
    bp_d = nc.declare_dram_parameter("b_proj", [C], F32, isOutput=False)
    # exp(bias) planes per head, transposed chunk-paired orientation:
    # expb [k_part 0:128, head, q 0:197 (k chunk 0) ++ q 0:197 (k chunk 1)]
    # (chunk 1 rows beyond k=196 are 1.0 and multiply unused garbage);
    # partition-major host layout so it loads as ONE contiguous DMA
    expb_d = nc.declare_dram_parameter("expb", [128, H * W2], BF16,
                                       isOutput=False)
    out_d = nc.declare_dram_parameter("out", [R, C], BF16, isOutput=True)

    with tile.TileContext(nc) as tc:
        with ExitStack() as ctx:
            _emit(ctx, tc, nc, x_d, wqkv_d, wp_d, bp_d, expb_d, out_d)
    nc.compile()
    return nc


def _emit(ctx, tc, nc, x_d, wqkv_d, wp_d, bp_d, expb_d, out_d):
    singles = ctx.enter_context(tc.tile_pool(name="singles", bufs=1))
    ps_pool = ctx.enter_context(tc.tile_pool(name="ps", bufs=4, space="PSUM"))
    av_ps = ctx.enter_context(tc.tile_pool(name="av_ps", bufs=4, space="PSUM"))
    probs_pool = ctx.enter_context(tc.tile_pool(name="probs", bufs=8))
    rec_pool = ctx.enter_context(tc.tile_pool(name="rec", bufs=2))
    out_pool = ctx.enter_context(tc.tile_pool(name="outp", bufs=4))

    KC = C // 128  # 6 contraction chunks

    # ---------------- load operands (already bf16 + transposed) ----------
    xT = []     # 6 x [128, R] bf16
    wqkvT = []  # 6 x [128, 1536] bf16 (q,k columns; v loads separately)
    for kc in range(KC):
        t = singles.tile([128, R], BF16, tag=f"xT{kc}", name=f"xT{kc}")
        nc.gpsimd.dma_start(out=t[:], in_=x_d[128 * kc:128 * (kc + 1), :])
        xT.append(t)
        w = singles.tile([128, 2 * C], BF16, tag=f"wqkvT{kc}", name=f"wqkvT{kc}")
        nc.sync.dma_start(out=w[:],
                          in_=wqkv_d[128 * kc:128 * (kc + 1), 0:2 * C])
        wqkvT.append(w)
    # v columns of w_qkv: one strided DMA filling all six chunk tiles'
    # v-column ranges (6 descriptors/partition instead of 6 DMA issues)
    vcols = singles.tile([128, KC, C], BF16, tag="wqkv_v", name="wqkv_v")
    nc.gpsimd.dma_start(
        out=vcols[:],
        in_=bass.AP(tensor=wqkv_d, offset=2 * C,
                    ap=[[3 * C, 128], [128 * 3 * C, KC], [1, C]]))
    # wp: one strided DMA for all 6 contraction chunks
    wp_all = singles.tile([128, KC, C], BF16, tag="wp_all", name="wp_all")
    nc.sync.dma_start(
        out=wp_all[:],
        in_=bass.AP(tensor=wp_d, offset=0,
                    ap=[[C, 128], [128 * C, KC], [1, C]]))

    bproj_bc = singles.tile([128, C], F32, tag="bproj")
    nc.gpsimd.dma_start(out=bproj_bc[:],
                        in_=bass.AP(tensor=bp_d, offset=0, ap=[[0, 128], [1, C]]))

    expb_all = singles.tile([128, H, W2], BF16, tag="expb", name="expb")
    nc.sync.dma_start(out=expb_all[:], in_=expb_d[:, :])
    expb = [expb_all[:, h, :] for h in range(H)]  # [128, 394] per head

    # ---------------- QKV ----------------
    NCHUNK = 4
    NW = R // NCHUNK  # 394 columns per psum tile

    # qk_sb[ft][ncol]: SEPARATE [128, 394] tiles per column chunk (2
    # batches each). Dependency tracking is tile-granular for matmul
    # operands, so per-chunk tiles let a wave's scores wait on exactly the
    # one copy they need instead of all four.
    qk_sb = [[None] * NCHUNK for _ in range(12)]

    def emit_qk(ft):
        # kc-outer, all four column tiles held across the contraction: the
        # first chunks compute while later weight DMA chunks still arrive
        # (only matters for the ramp pair ft 0/6)
        tiles = [ps_pool.tile([128, NW], F32, tag="ps", name=f"qkps{ncol}")
                 for ncol in range(NCHUNK)]
        for kc in range(KC):
            for ncol in range(NCHUNK):
                nc.tensor.matmul(
                    out=tiles[ncol][:],
                    lhsT=wqkvT[kc][:, 128 * ft:128 * (ft + 1)],
                    rhs=xT[kc][:, NW * ncol:NW * (ncol + 1)],
                    start=(kc == 0), stop=(kc == KC - 1))
        for ncol in range(NCHUNK):
            tg = (ft % 6) % 3 + (3 if ft >= 6 else 0)
            dst = singles.tile([128, NW], BF16, tag=f"qk{tg}_{ncol}",
                               name=f"qk{ft}_{ncol}")
            qk_sb[ft][ncol] = dst
            nc.vector.tensor_copy(out=dst[:], in_=tiles[ncol][:])

    def emit_qk_pair(fta, ftb, ncols):
        # q/k tile pair, GEMMs + copies interleaved per column chunk.
        # Copies run as scalar-Copy (Copy shares the Exp act table, so no
        # table reload) -- DVE carries the recip/norm/expb work instead.
        for ncol in ncols:
            for i, ft in enumerate((fta, ftb)):
                ps = ps_pool.tile([128, NW], F32, tag="ps", name="qkps")
                for kc in range(KC):
                    nc.tensor.matmul(
                        out=ps[:],
                        lhsT=wqkvT[kc][:, 128 * ft:128 * (ft + 1)],
                        rhs=xT[kc][:, NW * ncol:NW * (ncol + 1)],
                        start=(kc == 0), stop=(kc == KC - 1))
                tg = (ft % 6) % 3 + (3 if ft >= 6 else 0)
                dst = singles.tile([128, NW], BF16, tag=f"qk{tg}_{ncol}",
                                   name=f"qk{ft}_{ncol}")
                qk_sb[ft][ncol] = dst
                if (ncol + i) % 2 == 0:
                    nc.scalar.activation(out=dst[:], in_=ps[:], func=AF.Copy)
                else:
                    nc.vector.tensor_copy(out=dst[:], in_=ps[:])

    # v_aug[b][c]: [128, 12, 65] bf16 (col 64 = ones)
    v_aug = [[None, None] for _ in range(B)]

    def emit_v(batches):
        for b in batches:
            for cchunk, (r0, nr) in enumerate(((N * b, 128), (N * b + 128, N - 128))):
                dst = singles.tile([128, H, HD + 1], BF16, tag=f"v{b}_{cchunk}",
                                   name=f"v{b}_{cchunk}")
                v_aug[b][cchunk] = dst
                nc.vector.memset(dst[:, :, HD:HD + 1], 1.0)
                for nh in range(2):
                    ps = ps_pool.tile([128, 384], F32, tag="ps")
                    for kc in range(KC):
                        nc.tensor.matmul(
                            out=ps[:nr, :],
                            lhsT=xT[kc][:, r0:r0 + nr],
                            rhs=vcols[:, kc, 384 * nh:384 * (nh + 1)],
                            start=(kc == 0), stop=(kc == KC - 1))
                    # PSUM->SBUF casts split DVE / scalar-Copy (gpsimd
                    # cannot read PSUM); both engines are idle at this
                    # point in the schedule
                    if (b + cchunk + nh) % 2 == 0:
                        nc.vector.tensor_copy(
                            out=dst[:nr, 6 * nh:6 * (nh + 1), 0:HD],
                            in_=ps[:nr, :].rearrange("p (h d) -> p h d", h=6))
                    else:
                        nc.scalar.activation(
                            out=dst[:nr, 6 * nh:6 * (nh + 1), 0:HD],
                            in_=ps[:nr, :].rearrange("p (h d) -> p h d", h=6),
                            func=AF.Copy)

    # deferred normalize-multiply closures (two half-flushes per wave)
    pending_norm = []

    def flush_norm():
        while pending_norm:
            pending_norm.pop(0)()

    # attn output, transposed: 6 tiles [128, R] bf16 (pair p = heads 2p,2p+1)
    attn_outT = []
    for p in range(NPAIR):
        attn_outT.append(singles.tile([128, R], BF16, tag=f"aoT{p}",
                                      name=f"aoT{p}"))

    def emit_attention_pair(p, waves=(0, 1), mid_cb=None, end_cb=None,
                            eager=False):
        N1 = N - 128  # 69
        dst = attn_outT[p]
        for wave in waves:
            avs = []
            c0s = []

            def emit_scores(j):
                b = 4 * wave + j
                c0s.append(N * b)
                qt = qk_sb[p][b // 2]
                kt = qk_sb[6 + p][b // 2]
                o = N * (b % 2)
                qh = [qt[0:64, o:o + N], qt[64:128, o:o + N]]
                kh = [kt[0:64, o:o + N], kt[64:128, o:o + N]]
                phs = []
                for hh in range(2):
                    sth = ps_pool.tile([128, W2], F32, tag="ps",
                                       name=f"sth{hh}")
                    if SIM_SAFE:
                        nc.vector.memset(sth[64:128, N:W2], 0.0)
                    nc.tensor.matmul(out=sth[:, 0:N],
                                     lhsT=kh[hh][:, 0:128], rhs=qh[hh],
                                     start=True, stop=True)
                    nc.tensor.matmul(out=sth[0:N1, N:W2],
                                     lhsT=kh[hh][:, 128:N], rhs=qh[hh],
                                     start=True, stop=True)
                    ph = probs_pool.tile([128, W2], BF16, tag="probs")
                    nc.scalar.activation(out=ph[:], in_=sth[:], func=AF.Exp)
                    # expb multiply: all-SBUF bf16 fast DVE path, kept on
                    # DVE only -- on gpsimd it sat behind the broadcast in
                    # the in-order queue and stalled the AV matmuls
                    nc.vector.tensor_mul(out=ph[:], in0=ph[:],
                                         in1=expb[2 * p + hh])
                    phs.append(ph)
                return phs

            def emit_av(j, phs):
                b = 4 * wave + j
                av = av_ps.tile([HD + 1, W2], F32, tag="av")
                avs.append(av)
                for hh in range(2):
                    h = 2 * p + hh
                    nc.tensor.matmul(out=av[:, N * hh:N * hh + N],
                                     lhsT=v_aug[b][0][:, h, :],
                                     rhs=phs[hh][:, 0:N],
                                     start=True, stop=False)
                    nc.tensor.matmul(out=av[:, N * hh:N * hh + N],
                                     lhsT=v_aug[b][1][0:N1, h, :],
                                     rhs=phs[hh][0:N1, N:W2],
                                     start=False, stop=True)

            def norm_muls(js, rec_sb, avs=avs, c0s=c0s):
                for j in js:
                    o = W2 * j
                    nc.vector.tensor_mul(out=dst[0:64, c0s[j]:c0s[j] + N],
                                         in0=avs[j][0:HD, 0:N],
                                         in1=rec_sb[0:64, o:o + N])
                    nc.vector.tensor_mul(out=dst[64:128, c0s[j]:c0s[j] + N],
                                         in0=avs[j][0:HD, N:W2],
                                         in1=rec_sb[64:128, o + N:o + W2])

            def norm_prep():
                # softmax denominators: copy the 4 PSUM rows to SBUF on the
                # scalar engine (Copy shares the Exp table -> no reload; the
                # custom-DVE approx reciprocal reads garbage from PSUM),
                # then one packed ~18-bit reciprocal_approx_fast on DVE and
                # one gpsimd partition_broadcast. Runs eagerly at wave end;
                # only the 8 normalize multiplies are deferred.
                d4 = rec_pool.tile([1, 4 * W2], F32, tag="d4")
                for j in range(4):
                    nc.scalar.activation(out=d4[0:1, W2 * j:W2 * (j + 1)],
                                         in_=avs[j][HD:HD + 1, :],
                                         func=AF.Copy)
                rec4 = rec_pool.tile([1, 4 * W2], F32, tag="rec4")
                nc.vector.reciprocal_approx_fast(out=rec4[0:1, :],
                                                 in_=d4[0:1, :])
                rec_sb = rec_pool.tile([128, 4 * W2], F32, tag="rec_sb")
                nc.gpsimd.partition_broadcast(rec_sb[:], rec4[0:1, :])
                return rec_sb

            def norm_eager_j(j):
                # last pair: per-j chain so the proj tail isn't gated on a
                # whole-wave normalize (cuts ~4us off the tail)
                d1 = rec_pool.tile([1, W2], F32, tag="d1")
                nc.scalar.activation(out=d1[0:1, :],
                                     in_=avs[j][HD:HD + 1, :], func=AF.Copy)
                r1 = rec_pool.tile([1, W2], F32, tag="r1")
                nc.vector.reciprocal_approx_fast(out=r1[0:1, :], in_=d1[0:1, :])
                r1_sb = rec_pool.tile([128, W2], F32, tag="r1_sb")
                nc.gpsimd.partition_broadcast(r1_sb[:], r1[0:1, :])
                nc.vector.tensor_mul(out=dst[0:64, c0s[j]:c0s[j] + N],
                                     in0=avs[j][0:HD, 0:N],
                                     in1=r1_sb[0:64, 0:N])
                nc.vector.tensor_mul(out=dst[64:128, c0s[j]:c0s[j] + N],
                                     in0=avs[j][0:HD, N:W2],
                                     in1=r1_sb[64:128, N:W2])

            pending = [emit_scores(0), emit_scores(1)]
            # previous wave's first normalize half: frees av banks for this
            # wave's first AVs without stacking 8 multiplies on DVE at once
            if pending_norm:
                pending_norm.pop(0)()
            emit_av(0, pending[0])
            pending.append(emit_scores(2))
            if pending_norm:
                pending_norm.pop(0)()
            emit_av(1, pending[1])
            pending.append(emit_scores(3))
            if eager:
                norm_eager_j(0)
                norm_eager_j(1)
            emit_av(2, pending[2])
            emit_av(3, pending[3])
            if eager:
                norm_eager_j(2)
                norm_eager_j(3)
            else:
                rec_sb = norm_prep()
                pending_norm.append(
                    lambda r=rec_sb, f=norm_muls: f((0, 1), r))
                pending_norm.append(
                    lambda r=rec_sb, f=norm_muls: f((2, 3), r))
            if wave == waves[0] and mid_cb is not None:
                mid_cb()
        if end_cb is not None:
            end_cb()

    # ---------------- proj ----------------
    NRC = (R + 127) // 128  # 13 row chunks

    def emit_proj(rcs):
        for rc in rcs:
            r0 = 128 * rc
            nr = min(128, R - r0)
            ot = out_pool.tile([128, C], BF16, tag="out")
            for nh in range(2):
                ps = ps_pool.tile([128, 384], F32, tag="ps")
                for kc in range(KC):
                    nc.tensor.matmul(
                        out=ps[:nr, :],
                        lhsT=attn_outT[kc][:, r0:r0 + nr],
                        rhs=wp_all[:, kc, 384 * nh:384 * (nh + 1)],
                        start=(kc == 0), stop=(kc == KC - 1))
                nc.vector.tensor_add(out=ot[:nr, 384 * nh:384 * (nh + 1)],
                                     in0=ps[:nr, :],
                                     in1=bproj_bc[:nr, 384 * nh:384 * (nh + 1)])
            # one [*, 768] store per row chunk; alternate store queues so
            # descriptor issue parallelizes across the sync and gpsimd DGE
            eng = nc.sync if rc % 2 == 0 else nc.gpsimd
            eng.dma_start(out=out_d[r0:r0 + nr, :], in_=ot[:nr, :])

    # ---------------- emission schedule ----------------
    # ramp: q/k for pair 0, v for the first wave's batches
    emit_qk(0)
    emit_qk(6)
    emit_v(range(0, 4))

    def _mk_mid(pn, first):
        # between pair p's waves: (pair 0 only) v for wave-1 batches, then
        # the next pair's qk column chunks 0,1 (needed by its wave 0)
        def cb():
            if first:
                emit_v(range(4, 8))
            emit_qk_pair(pn, 6 + pn, ncols=(0, 1))
        return cb

    def _mk_end(pn):
        # after pair p's wave 1: next pair's qk chunks 2,3 (needed only by
        # its wave 1 -- plenty of slack)
        return lambda: emit_qk_pair(pn, 6 + pn, ncols=(2, 3))

    for p in range(NPAIR - 1):
        emit_attention_pair(p, mid_cb=_mk_mid(p + 1, p == 0),
                            end_cb=_mk_end(p + 1))

    # last pair: wave 0 (batches 0-3), then the proj row-chunks those
    # batches complete, then wave 1, then the rest -- shrinks the tail.
    # eager=True normalizes per-j so proj isn't gated on a wave-wide chain.
    emit_attention_pair(NPAIR - 1, waves=(0,), eager=True)
    flush_norm()
    emit_proj(range(0, 6))
    emit_attention_pair(NPAIR - 1, waves=(1,), eager=True)
    emit_proj(range(6, NRC))


_NC_CACHE = {}


def _get_nc():
    if "nc" not in _NC_CACHE:
        _NC_CACHE["nc"] = build_program()
    return _NC_CACHE["nc"]


def prep_aux(rpb_table, rel_idx):
    """Host-side prep: gather the bias from the two small aux inputs, lay it
    out per head PAIR in the kernel's transposed plane orientation
    [k_tok, q_tok*2] with zeroed CLS row/col, and exponentiate (bf16)."""
    import ml_dtypes
    bT = rpb_table[rel_idx.reshape(-1)].reshape(NP, NP, H)  # [q_idx, k_idx, h]
    bT = np.ascontiguousarray(bT.transpose(1, 0, 2))        # [k_idx, q_idx, h]
    bias0 = np.zeros((128, N, H), dtype=np.float32)
    bias0[1:128, 1:NP + 1, :] = bT[0:127]
    bias1 = np.zeros((128, N, H), dtype=np.float32)
    bias1[0:NP - 127, 1:NP + 1, :] = bT[127:NP]
    # partition-major [128, H, W2] so the device loads it as one DMA
    expb = np.zeros((128, H, W2), dtype=np.float32)
    for h in range(H):
        expb[:, h, 0:N] = np.exp(bias0[:, :, h])
        expb[:, h, N:W2] = np.exp(bias1[:, :, h])
    return expb.reshape(128, H * W2).astype(ml_dtypes.bfloat16)


def prep_weights(w_qkv, w_proj):
    """Host-side prep: transpose, fold the q scale into w_qkv, cast bf16."""
    import ml_dtypes
    wqkvT = np.array(w_qkv, dtype=np.float32).T.copy()
    wqkvT[:, 0:C] *= HD ** -0.5
    wpT = np.ascontiguousarray(np.asarray(w_proj, dtype=np.float32).T)
    return (wqkvT.astype(ml_dtypes.bfloat16), wpT.astype(ml_dtypes.bfloat16))


def make_in_maps(x, w_qkv, w_proj, b_proj, rpb_table, rel_idx):
    """Build the 8 per-core input maps (host prep: shard, transpose, bf16)."""
    import ml_dtypes
    x = np.asarray(x, dtype=np.float32)
    expb = prep_aux(
        np.asarray(rpb_table, dtype=np.float32), np.asarray(rel_idx).astype(np.int64))
    wqkvT, wpT = prep_weights(w_qkv, w_proj)
    bp = np.ascontiguousarray(np.asarray(b_proj, dtype=np.float32))
    xbf = x.astype(ml_dtypes.bfloat16)
    in_maps = []
    for c in range(NCORES):
        xT = np.ascontiguousarray(xbf[c * B:(c + 1) * B].reshape(R, C).T)
        in_maps.append({
            "xT": xT,
            "w_qkvT": wqkvT,
            "w_projT": wpT,
            "b_proj": bp,
            "expb": expb,
        })
    return in_maps


def kernel(x, w_qkv, w_proj, b_proj, rpb_table, rel_idx):
    from concourse.bass_utils import run_bass_kernel_spmd

    nc = _get_nc()
    in_maps = make_in_maps(x, w_qkv, w_proj, b_proj, rpb_table, rel_idx)
    res = run_bass_kernel_spmd(nc, in_maps, list(range(NCORES)))
    out = np.concatenate(
        [np.asarray(r["out"], dtype=np.float32).reshape(B, N, C)
         for r in res.results], axis=0)
    return out


# revision 30
# speedup vs baseline: 1.6944x; 1.0466x over previous
"""RPE (relative-position-bias) attention kernel for Trainium2, 8-core SPMD.

Full op (per reference):
  qkv = x @ w_qkv.T -> split q,k,v heads (H=12, hd=64), q *= hd**-0.5
  attn = q @ k.T ; attn[:, :, 1:, 1:] += rpb_table[rel_idx]  (per head)
  attn = softmax(attn, -1) ; out = (attn @ v) @ w_proj.T + b_proj

Sharding: data-parallel over batch. B=64 -> 8 batches per core. Weights
and bias-derived planes replicated to all cores. No collectives.

Per-core program (all matmuls bf16 operands, fp32 PSUM accumulation):
  - Inputs arrive bf16 and pre-transposed from the host: xT [768,1576],
    wqkvT [768,2304] (q columns pre-scaled by hd**-0.5), wpT [768,768].
  - qT,kT [768,1576] = w_chunk.T @ xT (transposed layout). v in natural
    layout [tokens, head, 65] with a ones column (softmax denominators
    fall out of the AV matmul for free).
  - The relative-position bias enters as exp(bias): probs = exp(s) *
    expb, expb planes host-precomputed bf16 in the transposed
    orientation [k_tok, q_tok] per head PAIR (two heads side by side,
    394 columns). exp runs on the scalar engine straight out of PSUM;
    the expb multiply runs on DVE in SBUF (bf16 fast path).
  - Heads are processed in pairs: score tiles [128,394] hold two heads.
  - Softmax normalization: denominators live in row 64 of the AV PSUM
    tile. Reciprocals run on DVE (keeps the scalar engine's activation
    table pinned to Exp -- the scalar-recip variant paid a 1.3us
    Exp<->Recip table reload per wave). The 4 reciprocal rows of a wave
    pack into one [1, 4*394] tile so a single gpsimd partition_broadcast
    per wave fans them out; the normalize multiplies run on DVE.
  - Engine balance: exp on scalar; qk PSUM->SBUF casts split between
    scalar (Copy shares the Exp table, no reload) and DVE; v casts on
    gpsimd; norm muls + expb muls + recips on DVE.
  - qk GEMMs for pair p+1 are emitted in two halves: column chunks 0,1
    between pair p's waves (needed by p+1 wave 0), chunks 2,3 after
    (needed only by p+1 wave 1). This keeps the PE fed across the wave
    boundary instead of bunching all 48 GEMMs at one point.
  - out = attn_outT.T @ wpT + b_proj  (fp32 output).
"""
import sys

sys.path.insert(0, '/opt/trn_rl_repo')

from contextlib import ExitStack

import numpy as np

import concourse.bass as bass
import concourse.bacc as bacc
import concourse.tile as tile
from concourse import mybir

# ---- problem dims (hardcoded per contract) ----
NCORES = 8
B_FULL = 64
B = B_FULL // NCORES     # 8 batches per core
N = 197                  # tokens (196 patches + CLS)
NP = 196
C = 768
H = 12
HD = 64
R = B * N                # 1576 rows per core
NPAIR = H // 2           # 6 head pairs
W2 = 2 * N               # 394 columns for a head pair

F32 = mybir.dt.float32
BF16 = mybir.dt.bfloat16
AF = mybir.ActivationFunctionType

import os
# CoreSim rejects reads of uninitialized PSUM; the exp deliberately reads a
# dead corner of the score tile (rows 69:128 of the chunk-1 columns, never
# consumed downstream). Sim runs memset it; hardware runs skip the cost.
SIM_SAFE = int(os.environ.get("KERNEL_SIM_SAFE", "0"))


def build_program():
    nc = bacc.Bacc("TRN2", target_bir_lowering=False, debug=False)

    x_d = nc.declare_dram_parameter("xT", [C, R], BF16, isOutput=False)
    wqkv_d = nc.declare_dram_parameter("w_qkvT", [C, 3 * C], BF16, isOutput=False)
    wp_d = nc.declare_dram_parameter("w_projT", [C, C], BF16, isOutput=False)
    bp_d = nc.declare_dram_parameter("b_proj", [C], F32, isOutput=False)
    # exp(bias) planes per head, transposed chunk-paired orientation:
    # expb [k_part 0:128, head, q 0:197 (k chunk 0) ++ q 0:197 (k chunk 1)]
    # (chunk 1 rows beyond k=196 are 1.0 and multiply unused garbage);
    # partition-major host layout so it loads as ONE contiguous DMA
    expb_d = nc.declare_dram_parameter("expb", [128, H * W2], BF16,
                                       isOutput=False)
    out_d = nc.declare_dram_parameter("out", [R, C], BF16, isOutput=True)

    with tile.TileContext(nc) as tc:
        with ExitStack() as ctx:
            _emit(ctx, tc, nc, x_d, wqkv_d, wp_d, bp_d, expb_d, out_d)
    nc.compile()
    return nc


def _emit(ctx, tc, nc, x_d, wqkv_d, wp_d, bp_d, expb_d, out_d):
    singles = ctx.enter_context(tc.tile_pool(name="singles", bufs=1))
    ps_pool = ctx.enter_context(tc.tile_pool(name="ps", bufs=4, space="PSUM"))
    av_ps = ctx.enter_context(tc.tile_pool(name="av_ps", bufs=4, space="PSUM"))
    probs_pool = ctx.enter_context(tc.tile_pool(name="probs", bufs=8))
    rec_pool = ctx.enter_context(tc.tile_pool(name="rec", bufs=2))
    out_pool = ctx.enter_context(tc.tile_pool(name="outp", bufs=4))

    KC = C // 128  # 6 contraction chunks

    # ---------------- load operands (already bf16 + transposed) ----------
    xT = []     # 6 x [128, R] bf16
    wqkvT = []  # 6 x [128, 1536] bf16 (q,k columns; v loads separately)
    for kc in range(KC):
        t = singles.tile([128, R], BF16, tag=f"xT{kc}", name=f"xT{kc}")
        nc.gpsimd.dma_start(out=t[:], in_=x_d[128 * kc:128 * (kc + 1), :])
        xT.append(t)
        w = singles.tile([128, 2 * C], BF16, tag=f"wqkvT{kc}", name=f"wqkvT{kc}")
        nc.sync.dma_start(out=w[:],
                          in_=wqkv_d[128 * kc:128 * (kc + 1), 0:2 * C])
        wqkvT.append(w)
    # remaining inputs issue from the (idle at startup) scalar DGE queue so the xT/wqkv chunk DMAs aren't stuck behind them.
    # v columns of w_qkv: one strided DMA filling all six chunk tiles'
    # v-column ranges (6 descriptors/partition instead of 6 DMA issues)
    vcols = singles.tile([128, KC, C], BF16, tag="wqkv_v", name="wqkv_v")
    nc.scalar.dma_start(
        out=vcols[:],
        in_=bass.AP(tensor=wqkv_d, offset=2 * C,
                    ap=[[3 * C, 128], [128 * 3 * C, KC], [1, C]]))
    # wp: one strided DMA for all 6 contraction chunks
    wp_all = singles.tile([128, KC, C], BF16, tag="wp_all", name="wp_all")
    nc.scalar.dma_start(
        out=wp_all[:],
        in_=bass.AP(tensor=wp_d, offset=0,
                    ap=[[C, 128], [128 * C, KC], [1, C]]))

    bproj_bc = singles.tile([128, C], F32, tag="bproj")
    nc.scalar.dma_start(out=bproj_bc[:],
                        in_=bass.AP(tensor=bp_d, offset=0, ap=[[0, 128], [1, C]]))

    expb_all = singles.tile([128, H, W2], BF16, tag="expb", name="expb")
    nc.scalar.dma_start(out=expb_all[:], in_=expb_d[:, :])
    expb = [expb_all[:, h, :] for h in range(H)]  # [128, 394] per head

    # ---------------- QKV ----------------
    NCHUNK = 4
    NW = R // NCHUNK  # 394 columns per psum tile

    # qk_sb[ft][ncol]: SEPARATE [128, 394] tiles per column chunk (2
    # batches each). Dependency tracking is tile-granular for matmul
    # operands, so per-chunk tiles let a wave's scores wait on exactly the
    # one copy they need instead of all four.
    qk_sb = [[None] * NCHUNK for _ in range(12)]

    def emit_qk(ft):
        # kc-outer, all four column tiles held across the contraction: the
        # first chunks compute while later weight DMA chunks still arrive
        # (only matters for the ramp pair ft 0/6)
        tiles = [ps_pool.tile([128, NW], F32, tag="ps", name=f"qkps{ncol}")
                 for ncol in range(NCHUNK)]
        for kc in range(KC):
            for ncol in range(NCHUNK):
                nc.tensor.matmul(
                    out=tiles[ncol][:],
                    lhsT=wqkvT[kc][:, 128 * ft:128 * (ft + 1)],
                    rhs=xT[kc][:, NW * ncol:NW * (ncol + 1)],
                    start=(kc == 0), stop=(kc == KC - 1))
        for ncol in range(NCHUNK):
            tg = (ft % 6) % 3 + (3 if ft >= 6 else 0)
            dst = singles.tile([128, NW], BF16, tag=f"qk{tg}_{ncol}",
                               name=f"qk{ft}_{ncol}")
            qk_sb[ft][ncol] = dst
            nc.vector.tensor_copy(out=dst[:], in_=tiles[ncol][:])

    def emit_qk_pair(fta, ftb, ncols):
        # q/k tile pair, GEMMs + copies interleaved per column chunk.
        # Copies run as scalar-Copy (Copy shares the Exp act table, so no
        # table reload) -- DVE carries the recip/norm/expb work instead.
        for ncol in ncols:
            for i, ft in enumerate((fta, ftb)):
                ps = ps_pool.tile([128, NW], F32, tag="ps", name="qkps")
                for kc in range(KC):
                    nc.tensor.matmul(
                        out=ps[:],
                        lhsT=wqkvT[kc][:, 128 * ft:128 * (ft + 1)],
                        rhs=xT[kc][:, NW * ncol:NW * (ncol + 1)],
                        start=(kc == 0), stop=(kc == KC - 1))
                tg = (ft % 6) % 3 + (3 if ft >= 6 else 0)
                dst = singles.tile([128, NW], BF16, tag=f"qk{tg}_{ncol}",
                                   name=f"qk{ft}_{ncol}")
                qk_sb[ft][ncol] = dst
                if (ncol + i) % 2 == 0:
                    nc.scalar.activation(out=dst[:], in_=ps[:], func=AF.Copy)
                else:
                    nc.vector.tensor_copy(out=dst[:], in_=ps[:])

    # v_aug[b][c]: [128, 12, 65] bf16 (col 64 = ones)
    v_aug = [[None, None] for _ in range(B)]

    def emit_v(batches):
        for b in batches:
            for cchunk, (r0, nr) in enumerate(((N * b, 128), (N * b + 128, N - 128))):
                dst = singles.tile([128, H, HD + 1], BF16, tag=f"v{b}_{cchunk}",
                                   name=f"v{b}_{cchunk}")
                v_aug[b][cchunk] = dst
                nc.vector.memset(dst[:, :, HD:HD + 1], 1.0)
                for nh in range(2):
                    ps = ps_pool.tile([128, 384], F32, tag="ps")
                    for kc in range(KC):
                        nc.tensor.matmul(
                            out=ps[:nr, :],
                            lhsT=xT[kc][:, r0:r0 + nr],
                            rhs=vcols[:, kc, 384 * nh:384 * (nh + 1)],
                            start=(kc == 0), stop=(kc == KC - 1))
                    # PSUM->SBUF casts split DVE / scalar-Copy (gpsimd
                    # cannot read PSUM); both engines are idle at this
                    # point in the schedule
                    if (b + cchunk + nh) % 2 == 0:
                        nc.vector.tensor_copy(
                            out=dst[:nr, 6 * nh:6 * (nh + 1), 0:HD],
                            in_=ps[:nr, :].rearrange("p (h d) -> p h d", h=6))
                    else:
                        nc.scalar.activation(
                            out=dst[:nr, 6 * nh:6 * (nh + 1), 0:HD],
                            in_=ps[:nr, :].rearrange("p (h d) -> p h d", h=6),
                            func=AF.Copy)

    # deferred normalize-multiply closures (two half-flushes per wave)
    pending_norm = []

    def flush_norm():
        while pending_norm:
            pending_norm.pop(0)()

    # attn output, transposed: 6 tiles [128, R] bf16 (pair p = heads 2p,2p+1)
    attn_outT = []
    for p in range(NPAIR):
        attn_outT.append(singles.tile([128, R], BF16, tag=f"aoT{p}",
                                      name=f"aoT{p}"))

    def emit_attention_pair(p, waves=(0, 1), fillers=(None, None),
                            eager=False):
        N1 = N - 128  # 69
        dst = attn_outT[p]
        for wave in waves:
            avs = []
            c0s = []

            def emit_scores(j):
                b = 4 * wave + j
                c0s.append(N * b)
                qt = qk_sb[p][b // 2]
                kt = qk_sb[6 + p][b // 2]
                o = N * (b % 2)
                qh = [qt[0:64, o:o + N], qt[64:128, o:o + N]]
                kh = [kt[0:64, o:o + N], kt[64:128, o:o + N]]
                phs = []
                for hh in range(2):
                    sth = ps_pool.tile([128, W2], F32, tag="ps",
                                       name=f"sth{hh}")
                    if SIM_SAFE:
                        nc.vector.memset(sth[64:128, N:W2], 0.0)
                    nc.tensor.matmul(out=sth[:, 0:N],
                                     lhsT=kh[hh][:, 0:128], rhs=qh[hh],
                                     start=True, stop=True)
                    nc.tensor.matmul(out=sth[0:N1, N:W2],
                                     lhsT=kh[hh][:, 128:N], rhs=qh[hh],
                                     start=True, stop=True)
                    ph = probs_pool.tile([128, W2], BF16, tag="probs")
                    nc.scalar.activation(out=ph[:], in_=sth[:], func=AF.Exp)
                    # expb multiply: all-SBUF bf16 fast DVE path, kept on
                    # DVE only -- on gpsimd it sat behind the broadcast in
                    # the in-order queue and stalled the AV matmuls
                    nc.vector.tensor_mul(out=ph[:], in0=ph[:],
                                         in1=expb[2 * p + hh])
                    phs.append(ph)
                return phs

            def emit_av(j, phs):
                b = 4 * wave + j
                av = av_ps.tile([HD + 1, W2], F32, tag="av")
                avs.append(av)
                for hh in range(2):
                    h = 2 * p + hh
                    nc.tensor.matmul(out=av[:, N * hh:N * hh + N],
                                     lhsT=v_aug[b][0][:, h, :],
                                     rhs=phs[hh][:, 0:N],
                                     start=True, stop=False)
                    nc.tensor.matmul(out=av[:, N * hh:N * hh + N],
                                     lhsT=v_aug[b][1][0:N1, h, :],
                                     rhs=phs[hh][0:N1, N:W2],
                                     start=False, stop=True)

            def norm_muls(js, rec_sb, avs=avs, c0s=c0s):
                for j in js:
                    o = W2 * j
                    nc.vector.tensor_mul(out=dst[0:64, c0s[j]:c0s[j] + N],
                                         in0=avs[j][0:HD, 0:N],
                                         in1=rec_sb[0:64, o:o + N])
                    nc.vector.tensor_mul(out=dst[64:128, c0s[j]:c0s[j] + N],
                                         in0=avs[j][0:HD, N:W2],
                                         in1=rec_sb[64:128, o + N:o + W2])

            def norm_prep():
                # softmax denominators: copy the 4 PSUM rows to SBUF on the
                # scalar engine (Copy shares the Exp table -> no reload; the
                # custom-DVE approx reciprocal reads garbage from PSUM),
                # then one packed ~18-bit reciprocal_approx_fast on DVE and
                # one gpsimd partition_broadcast. Runs eagerly at wave end;
                # only the 8 normalize multiplies are deferred.
                d4 = rec_pool.tile([1, 4 * W2], F32, tag="d4")
                for j in range(4):
                    nc.scalar.activation(out=d4[0:1, W2 * j:W2 * (j + 1)],
                                         in_=avs[j][HD:HD + 1, :],
                                         func=AF.Copy)
                rec4 = rec_pool.tile([1, 4 * W2], F32, tag="rec4")
                nc.vector.reciprocal_approx_fast(out=rec4[0:1, :],
                                                 in_=d4[0:1, :])
                rec_sb = rec_pool.tile([128, 4 * W2], F32, tag="rec_sb")
                nc.gpsimd.partition_broadcast(rec_sb[:], rec4[0:1, :])
                return rec_sb

            def norm_eager_j(j):
                # last pair: per-j chain so the proj tail isn't gated on a
                # whole-wave normalize (cuts ~4us off the tail)
                d1 = rec_pool.tile([1, W2], F32, tag="d1")
                nc.scalar.activation(out=d1[0:1, :],
                                     in_=avs[j][HD:HD + 1, :], func=AF.Copy)
                r1 = rec_pool.tile([1, W2], F32, tag="r1")
                nc.vector.reciprocal_approx_fast(out=r1[0:1, :], in_=d1[0:1, :])
                r1_sb = rec_pool.tile([128, W2], F32, tag="r1_sb")
                nc.gpsimd.partition_broadcast(r1_sb[:], r1[0:1, :])
                nc.vector.tensor_mul(out=dst[0:64, c0s[j]:c0s[j] + N],
                                     in0=avs[j][0:HD, 0:N],
                                     in1=r1_sb[0:64, 0:N])
                nc.vector.tensor_mul(out=dst[64:128, c0s[j]:c0s[j] + N],
                                     in0=avs[j][0:HD, N:W2],
                                     in1=r1_sb[64:128, N:W2])

            pending = [emit_scores(0), emit_scores(1)]
            # filler PE work (next pair's qk GEMMs / v GEMMs / proj) lands
            # HERE -- after this wave's first scores, so the PE chews on it
            # exactly while the scalar engine works through the exps the
            # first AVs depend on (was a ~1.5us PE stall per wave)
            filler = fillers[wave] if wave < len(fillers) else None
            if filler is not None:
                filler()
            # previous wave's first normalize half: frees av banks for this
            # wave's first AVs without stacking 8 multiplies on DVE at once
            if pending_norm:
                pending_norm.pop(0)()
            emit_av(0, pending[0])
            pending.append(emit_scores(2))
            if pending_norm:
                pending_norm.pop(0)()
            emit_av(1, pending[1])
            pending.append(emit_scores(3))
            if eager:
                norm_eager_j(0)
                norm_eager_j(1)
            emit_av(2, pending[2])
            emit_av(3, pending[3])
            if eager:
                norm_eager_j(2)
                norm_eager_j(3)
            else:
                rec_sb = norm_prep()
                pending_norm.append(
                    lambda r=rec_sb, f=norm_muls: f((0, 1), r))
                pending_norm.append(
                    lambda r=rec_sb, f=norm_muls: f((2, 3), r))

    # ---------------- proj ----------------
    NRC = (R + 127) // 128  # 13 row chunks

    def emit_proj(rcs):
        for rc in rcs:
            r0 = 128 * rc
            nr = min(128, R - r0)
            ot = out_pool.tile([128, C], BF16, tag="out")
            for nh in range(2):
                ps = ps_pool.tile([128, 384], F32, tag="ps")
                for kc in range(KC):
                    nc.tensor.matmul(
                        out=ps[:nr, :],
                        lhsT=attn_outT[kc][:, r0:r0 + nr],
                        rhs=wp_all[:, kc, 384 * nh:384 * (nh + 1)],
                        start=(kc == 0), stop=(kc == KC - 1))
                nc.vector.tensor_add(out=ot[:nr, 384 * nh:384 * (nh + 1)],
                                     in0=ps[:nr, :],
                                     in1=bproj_bc[:nr, 384 * nh:384 * (nh + 1)])
            # one [*, 768] store per row chunk; alternate store queues so
            # descriptor issue parallelizes across the sync and gpsimd DGE
            eng = nc.sync if rc % 2 == 0 else nc.gpsimd
            eng.dma_start(out=out_d[r0:r0 + nr, :], in_=ot[:nr, :])

    # ---------------- emission schedule ----------------
    # ramp: q/k for pair 0 (kc-outer so compute starts while weight DMA
    # chunks still arrive), v for the first wave's batches
    emit_qk(0)
    emit_qk(6)
    emit_v(range(0, 4))

    def _qk(pn, ncols):
        return lambda: emit_qk_pair(pn, 6 + pn, ncols=ncols)

    # per pair p: wave-0 filler = p's own qk chunks 2,3 (needed by wave 1),
    # wave-1 filler = next pair's chunks 0,1 (needed by its wave 0).
    # pair 0's wave-0 filler is the remaining v GEMMs instead (its chunks
    # 2,3 came from the ramp).
    for p in range(NPAIR - 1):
        fa = (lambda: emit_v(range(4, 8))) if p == 0 else _qk(p, (2, 3))
        emit_attention_pair(p, fillers=(fa, _qk(p + 1, (0, 1))))

    # last pair: wave 0 (batches 0-3) with its chunks 2,3 as filler, then
    # wave 1 with the first proj row-chunks as filler, then the rest.
    # eager=True normalizes per-j so proj isn't gated on a wave-wide chain.
    emit_attention_pair(NPAIR - 1, waves=(0,), eager=True,
                        fillers=(_qk(NPAIR - 1, (2, 3)),))
    emit_attention_pair(NPAIR - 1, waves=(1,), eager=True,
                        fillers=(None, lambda: emit_proj(range(0, 6))))
    emit_proj(range(6, NRC))


_NC_CACHE = {}


def _get_nc():
    if "nc" not in _NC_CACHE:
        _NC_CACHE["nc"] = build_program()
    return _NC_CACHE["nc"]


def prep_aux(rpb_table, rel_idx):
    """Host-side prep: gather the bias from the two small aux inputs, lay it
    out per head PAIR in the kernel's transposed plane orientation
    [k_tok, q_tok*2] with zeroed CLS row/col, and exponentiate (bf16)."""
    import ml_dtypes
    bT = rpb_table[rel_idx.reshape(-1)].reshape(NP, NP, H)  # [q_idx, k_idx, h]
    bT = np.ascontiguousarray(bT.transpose(1, 0, 2))        # [k_idx, q_idx, h]
    bias0 = np.zeros((128, N, H), dtype=np.float32)
    bias0[1:128, 1:NP + 1, :] = bT[0:127]
    bias1 = np.zeros((128, N, H), dtype=np.float32)
    bias1[0:NP - 127, 1:NP + 1, :] = bT[127:NP]
    # partition-major [128, H, W2] so the device loads it as one DMA
    expb = np.zeros((128, H, W2), dtype=np.float32)
    for h in range(H):
        expb[:, h, 0:N] = np.exp(bias0[:, :, h])
        expb[:, h, N:W2] = np.exp(bias1[:, :, h])
    return expb.reshape(128, H * W2).astype(ml_dtypes.bfloat16)


def prep_weights(w_qkv, w_proj):
    """Host-side prep: transpose, fold the q scale into w_qkv, cast bf16."""
    import ml_dtypes
    wqkvT = np.array(w_qkv, dtype=np.float32).T.copy()
    wqkvT[:, 0:C] *= HD ** -0.5
    wpT = np.ascontiguousarray(np.asarray(w_proj, dtype=np.float32).T)
    return (wqkvT.astype(ml_dtypes.bfloat16), wpT.astype(ml_dtypes.bfloat16))


def make_in_maps(x, w_qkv, w_proj, b_proj, rpb_table, rel_idx):
    """Build the 8 per-core input maps (host prep: shard, transpose, bf16)."""
    import ml_dtypes
    x = np.asarray(x, dtype=np.float32)
    expb = prep_aux(
        np.asarray(rpb_table, dtype=np.float32), np.asarray(rel_idx).astype(np.int64))
    wqkvT, wpT = prep_weights(w_qkv, w_proj)
    bp = np.ascontiguousarray(np.asarray(b_proj, dtype=np.float32))
    xbf = x.astype(ml_dtypes.bfloat16)
    in_maps = []
    for c in range(NCORES):
        xT = np.ascontiguousarray(xbf[c * B:(c + 1) * B].reshape(R, C).T)
        in_maps.append({
            "xT": xT,
            "w_qkvT": wqkvT,
            "w_projT": wpT,
            "b_proj": bp,
            "expb": expb,
        })
    return in_maps


def kernel(x, w_qkv, w_proj, b_proj, rpb_table, rel_idx):
    from concourse.bass_utils import run_bass_kernel_spmd

    nc = _get_nc()
    in_maps = make_in_maps(x, w_qkv, w_proj, b_proj, rpb_table, rel_idx)
    res = run_bass_kernel_spmd(nc, in_maps, list(range(NCORES)))
    out = np.concatenate(
        [np.asarray(r["out"], dtype=np.float32).reshape(B, N, C)
         for r in res.results], axis=0)
    return out


# revision 32
# speedup vs baseline: 1.7188x; 1.0144x over previous
"""RPE (relative-position-bias) attention kernel for Trainium2, 8-core SPMD.

Full op (per reference):
  qkv = x @ w_qkv.T -> split q,k,v heads (H=12, hd=64), q *= hd**-0.5
  attn = q @ k.T ; attn[:, :, 1:, 1:] += rpb_table[rel_idx]  (per head)
  attn = softmax(attn, -1) ; out = (attn @ v) @ w_proj.T + b_proj

Sharding: data-parallel over batch. B=64 -> 8 batches per core. Weights
and bias-derived planes replicated to all cores. No collectives.

Per-core program (all matmuls bf16 operands, fp32 PSUM accumulation):
  - Inputs arrive bf16 and pre-transposed from the host: xT [768,1576],
    wqkvT [768,2304] (q columns pre-scaled by hd**-0.5), wpT [768,768].
  - qT,kT [768,1576] = w_chunk.T @ xT (transposed layout). v in natural
    layout [tokens, head, 65] with a ones column (softmax denominators
    fall out of the AV matmul for free).
  - The relative-position bias enters as exp(bias): probs = exp(s) *
    expb, expb planes host-precomputed bf16 in the transposed
    orientation [k_tok, q_tok] per head PAIR (two heads side by side,
    394 columns). exp runs on the scalar engine straight out of PSUM;
    the expb multiply runs on DVE in SBUF (bf16 fast path).
  - Heads are processed in pairs: score tiles [128,394] hold two heads.
  - Softmax normalization: denominators live in row 64 of the AV PSUM
    tile. Reciprocals run on DVE (keeps the scalar engine's activation
    table pinned to Exp -- the scalar-recip variant paid a 1.3us
    Exp<->Recip table reload per wave). The 4 reciprocal rows of a wave
    pack into one [1, 4*394] tile so a single gpsimd partition_broadcast
    per wave fans them out; the normalize multiplies run on DVE.
  - Engine balance: exp on scalar; qk PSUM->SBUF casts split between
    scalar (Copy shares the Exp table, no reload) and DVE; v casts on
    gpsimd; norm muls + expb muls + recips on DVE.
  - qk GEMMs for pair p+1 are emitted in two halves: column chunks 0,1
    between pair p's waves (needed by p+1 wave 0), chunks 2,3 after
    (needed only by p+1 wave 1). This keeps the PE fed across the wave
    boundary instead of bunching all 48 GEMMs at one point.
  - out = attn_outT.T @ wpT + b_proj  (fp32 output).
"""
import sys

sys.path.insert(0, '/opt/trn_rl_repo')

from contextlib import ExitStack

import numpy as np

import concourse.bass as bass
import concourse.bacc as bacc
import concourse.tile as tile
from concourse import mybir

# ---- problem dims (hardcoded per contract) ----
NCORES = 8
B_FULL = 64
B = B_FULL // NCORES     # 8 batches per core
N = 197                  # tokens (196 patches + CLS)
NP = 196
C = 768
H = 12
HD = 64
R = B * N                # 1576 rows per core
NPAIR = H // 2           # 6 head pairs
W2 = 2 * N               # 394 columns for a head pair

F32 = mybir.dt.float32
BF16 = mybir.dt.bfloat16
AF = mybir.ActivationFunctionType

import os
# CoreSim rejects reads of uninitialized PSUM; the exp deliberately reads a
# dead corner of the score tile (rows 69:128 of the chunk-1 columns, never
# consumed downstream). Sim runs memset it; hardware runs skip the cost.
SIM_SAFE = int(os.environ.get("KERNEL_SIM_SAFE", "0"))


def build_program():
    nc = bacc.Bacc("TRN2", target_bir_lowering=False, debug=False)

    x_d = nc.declare_dram_parameter("xT", [C, R], BF16, isOutput=False)
    wqkv_d = nc.declare_dram_parameter("w_qkvT", [C, 3 * C], BF16, isOutput=False)
    wp_d = nc.declare_dram_parameter("w_projT", [C, C], BF16, isOutput=False)
    bp_d = nc.declare_dram_parameter("b_proj", [C], F32, isOutput=False)
    # exp(bias) planes per head, transposed chunk-paired orientation:
    # expb [k_part 0:128, head, q 0:197 (k chunk 0) ++ q 0:197 (k chunk 1)]
    # (chunk 1 rows beyond k=196 are 1.0 and multiply unused garbage);
    # partition-major host layout so it loads as ONE contiguous DMA
    expb_d = nc.declare_dram_parameter("expb", [128, H * W2], BF16,
                                       isOutput=False)
    out_d = nc.declare_dram_parameter("out", [R, C], BF16, isOutput=True)

    with tile.TileContext(nc) as tc:
        with ExitStack() as ctx:
            _emit(ctx, tc, nc, x_d, wqkv_d, wp_d, bp_d, expb_d, out_d)
    nc.compile()
    return nc


def _emit(ctx, tc, nc, x_d, wqkv_d, wp_d, bp_d, expb_d, out_d):
    singles = ctx.enter_context(tc.tile_pool(name="singles", bufs=1))
    ps_pool = ctx.enter_context(tc.tile_pool(name="ps", bufs=4, space="PSUM"))
    av_ps = ctx.enter_context(tc.tile_pool(name="av_ps", bufs=4, space="PSUM"))
    probs_pool = ctx.enter_context(tc.tile_pool(name="probs", bufs=8))
    rec_pool = ctx.enter_context(tc.tile_pool(name="rec", bufs=2))
    out_pool = ctx.enter_context(tc.tile_pool(name="outp", bufs=4))

    KC = C // 128  # 6 contraction chunks

    # ---------------- load operands (already bf16 + transposed) ----------
    xT = []     # 6 x [128, R] bf16
    wqkvT = []  # 6 x [128, 1536] bf16 (q,k columns; v loads separately)
    for kc in range(KC):
        t = singles.tile([128, R], BF16, tag=f"xT{kc}", name=f"xT{kc}")
        nc.gpsimd.dma_start(out=t[:], in_=x_d[128 * kc:128 * (kc + 1), :])
        xT.append(t)
        w = singles.tile([128, 2 * C], BF16, tag=f"wqkvT{kc}", name=f"wqkvT{kc}")
        nc.sync.dma_start(out=w[:],
                          in_=wqkv_d[128 * kc:128 * (kc + 1), 0:2 * C])
        wqkvT.append(w)
    # remaining inputs issue from the (idle at startup) scalar DGE queue so the xT/wqkv chunk DMAs aren't stuck behind them.
    # v columns of w_qkv: one strided DMA filling all six chunk tiles'
    # v-column ranges (6 descriptors/partition instead of 6 DMA issues)
    vcols = singles.tile([128, KC, C], BF16, tag="wqkv_v", name="wqkv_v")
    nc.scalar.dma_start(
        out=vcols[:],
        in_=bass.AP(tensor=wqkv_d, offset=2 * C,
                    ap=[[3 * C, 128], [128 * 3 * C, KC], [1, C]]))
    # wp: one strided DMA for all 6 contraction chunks
    wp_all = singles.tile([128, KC, C], BF16, tag="wp_all", name="wp_all")
    nc.scalar.dma_start(
        out=wp_all[:],
        in_=bass.AP(tensor=wp_d, offset=0,
                    ap=[[C, 128], [128 * C, KC], [1, C]]))

    bproj_bc = singles.tile([128, C], F32, tag="bproj")
    nc.scalar.dma_start(out=bproj_bc[:],
                        in_=bass.AP(tensor=bp_d, offset=0, ap=[[0, 128], [1, C]]))

    expb_all = singles.tile([128, H, W2], BF16, tag="expb", name="expb")
    nc.scalar.dma_start(out=expb_all[:], in_=expb_d[:, :])
    expb = [expb_all[:, h, :] for h in range(H)]  # [128, 394] per head

    # ---------------- QKV ----------------
    NCHUNK = 4
    NW = R // NCHUNK  # 394 columns per psum tile

    # qk_sb[ft][ncol]: SEPARATE [128, 394] tiles per column chunk (2
    # batches each). Dependency tracking is tile-granular for matmul
    # operands, so per-chunk tiles let a wave's scores wait on exactly the
    # one copy they need instead of all four.
    qk_sb = [[None] * NCHUNK for _ in range(12)]

    def emit_qk(ft):
        # kc-outer, all four column tiles held across the contraction: the
        # first chunks compute while later weight DMA chunks still arrive
        # (only matters for the ramp pair ft 0/6)
        tiles = [ps_pool.tile([128, NW], F32, tag="ps", name=f"qkps{ncol}")
                 for ncol in range(NCHUNK)]
        for kc in range(KC):
            for ncol in range(NCHUNK):
                nc.tensor.matmul(
                    out=tiles[ncol][:],
                    lhsT=wqkvT[kc][:, 128 * ft:128 * (ft + 1)],
                    rhs=xT[kc][:, NW * ncol:NW * (ncol + 1)],
                    start=(kc == 0), stop=(kc == KC - 1))
        for ncol in range(NCHUNK):
            tg = (ft % 6) % 3 + (3 if ft >= 6 else 0)
            dst = singles.tile([128, NW], BF16, tag=f"qk{tg}_{ncol}",
                               name=f"qk{ft}_{ncol}")
            qk_sb[ft][ncol] = dst
            nc.vector.tensor_copy(out=dst[:], in_=tiles[ncol][:])

    def emit_qk_pair(fta, ftb, ncols):
        # q/k tile pair, GEMMs + copies interleaved per column chunk.
        # Copies run as scalar-Copy (Copy shares the Exp act table, so no
        # table reload) -- DVE carries the recip/norm/expb work instead.
        for ncol in ncols:
            for i, ft in enumerate((fta, ftb)):
                ps = ps_pool.tile([128, NW], F32, tag="ps", name="qkps")
                for kc in range(KC):
                    nc.tensor.matmul(
                        out=ps[:],
                        lhsT=wqkvT[kc][:, 128 * ft:128 * (ft + 1)],
                        rhs=xT[kc][:, NW * ncol:NW * (ncol + 1)],
                        start=(kc == 0), stop=(kc == KC - 1))
                tg = (ft % 6) % 3 + (3 if ft >= 6 else 0)
                dst = singles.tile([128, NW], BF16, tag=f"qk{tg}_{ncol}",
                                   name=f"qk{ft}_{ncol}")
                qk_sb[ft][ncol] = dst
                if (ncol + i) % 2 == 0:
                    nc.scalar.activation(out=dst[:], in_=ps[:], func=AF.Copy)
                else:
                    nc.vector.tensor_copy(out=dst[:], in_=ps[:])

    # v_aug[b][c]: [128, 12, 65] bf16 (col 64 = ones)
    v_aug = [[None, None] for _ in range(B)]

    def emit_v(batches):
        for b in batches:
            for cchunk, (r0, nr) in enumerate(((N * b, 128), (N * b + 128, N - 128))):
                dst = singles.tile([128, H, HD + 1], BF16, tag=f"v{b}_{cchunk}",
                                   name=f"v{b}_{cchunk}")
                v_aug[b][cchunk] = dst
                nc.vector.memset(dst[:, :, HD:HD + 1], 1.0)
                for nh in range(2):
                    ps = ps_pool.tile([128, 384], F32, tag="ps")
                    for kc in range(KC):
                        nc.tensor.matmul(
                            out=ps[:nr, :],
                            lhsT=xT[kc][:, r0:r0 + nr],
                            rhs=vcols[:, kc, 384 * nh:384 * (nh + 1)],
                            start=(kc == 0), stop=(kc == KC - 1))
                    # PSUM->SBUF casts split DVE / scalar-Copy (gpsimd
                    # cannot read PSUM); both engines are idle at this
                    # point in the schedule
                    if (b + cchunk + nh) % 2 == 0:
                        nc.vector.tensor_copy(
                            out=dst[:nr, 6 * nh:6 * (nh + 1), 0:HD],
                            in_=ps[:nr, :].rearrange("p (h d) -> p h d", h=6))
                    else:
                        nc.scalar.activation(
                            out=dst[:nr, 6 * nh:6 * (nh + 1), 0:HD],
                            in_=ps[:nr, :].rearrange("p (h d) -> p h d", h=6),
                            func=AF.Copy)

    # deferred normalize-multiply closures (two half-flushes per wave)
    pending_norm = []

    def flush_norm():
        while pending_norm:
            pending_norm.pop(0)()

    # attn output, transposed: 6 tiles [128, R] bf16 (pair p = heads 2p,2p+1)
    attn_outT = []
    for p in range(NPAIR):
        attn_outT.append(singles.tile([128, R], BF16, tag=f"aoT{p}",
                                      name=f"aoT{p}"))

    def emit_attention_pair(p, waves=(0, 1), fillers=(None, None),
                            eager=False):
        N1 = N - 128  # 69
        dst = attn_outT[p]
        for wave in waves:
            avs = []
            c0s = []

            def emit_scores(j):
                b = 4 * wave + j
                c0s.append(N * b)
                qt = qk_sb[p][b // 2]
                kt = qk_sb[6 + p][b // 2]
                o = N * (b % 2)
                qh = [qt[0:64, o:o + N], qt[64:128, o:o + N]]
                kh = [kt[0:64, o:o + N], kt[64:128, o:o + N]]
                phs = []
                for hh in range(2):
                    sth = ps_pool.tile([128, W2], F32, tag="ps",
                                       name=f"sth{hh}")
                    if SIM_SAFE:
                        nc.vector.memset(sth[64:128, N:W2], 0.0)
                    nc.tensor.matmul(out=sth[:, 0:N],
                                     lhsT=kh[hh][:, 0:128], rhs=qh[hh],
                                     start=True, stop=True)
                    nc.tensor.matmul(out=sth[0:N1, N:W2],
                                     lhsT=kh[hh][:, 128:N], rhs=qh[hh],
                                     start=True, stop=True)
                    ph = probs_pool.tile([128, W2], BF16, tag="probs")
                    nc.scalar.activation(out=ph[:], in_=sth[:], func=AF.Exp)
                    # expb multiply: all-SBUF bf16 fast DVE path, kept on
                    # DVE only -- on gpsimd it sat behind the broadcast in
                    # the in-order queue and stalled the AV matmuls
                    nc.vector.tensor_mul(out=ph[:], in0=ph[:],
                                         in1=expb[2 * p + hh])
                    phs.append(ph)
                return phs

            def emit_av(j, phs):
                b = 4 * wave + j
                av = av_ps.tile([HD + 1, W2], F32, tag="av")
                avs.append(av)
                for hh in range(2):
                    h = 2 * p + hh
                    nc.tensor.matmul(out=av[:, N * hh:N * hh + N],
                                     lhsT=v_aug[b][0][:, h, :],
                                     rhs=phs[hh][:, 0:N],
                                     start=True, stop=False)
                    nc.tensor.matmul(out=av[:, N * hh:N * hh + N],
                                     lhsT=v_aug[b][1][0:N1, h, :],
                                     rhs=phs[hh][0:N1, N:W2],
                                     start=False, stop=True)

            def norm_muls(js, rec_sb, avs=avs, c0s=c0s):
                for j in js:
                    o = W2 * (j % 2)
                    nc.vector.tensor_mul(out=dst[0:64, c0s[j]:c0s[j] + N],
                                         in0=avs[j][0:HD, 0:N],
                                         in1=rec_sb[0:64, o:o + N])
                    nc.vector.tensor_mul(out=dst[64:128, c0s[j]:c0s[j] + N],
                                         in0=avs[j][0:HD, N:W2],
                                         in1=rec_sb[64:128, o + N:o + W2])

            def norm_prep(js):
                # softmax denominators for two j's: copy the PSUM rows to
                # SBUF on the scalar engine (Copy shares the Exp table -> no
                # reload; the custom-DVE approx reciprocal reads garbage
                # from PSUM), then a packed ~18-bit reciprocal_approx_fast
                # on DVE and a gpsimd partition_broadcast. Emitted in two
                # halves mid-wave so the chain finishes before the next
                # wave needs the av banks; only the multiplies are deferred.
                d2 = rec_pool.tile([1, 2 * W2], F32, tag="d2")
                for i, j in enumerate(js):
                    nc.scalar.activation(out=d2[0:1, W2 * i:W2 * (i + 1)],
                                         in_=avs[j][HD:HD + 1, :],
                                         func=AF.Copy)
                rec2 = rec_pool.tile([1, 2 * W2], F32, tag="rec2")
                nc.vector.reciprocal_approx_fast(out=rec2[0:1, :],
                                                 in_=d2[0:1, :])
                rec_sb = rec_pool.tile([128, 2 * W2], F32, tag="rec_sb2")
                nc.gpsimd.partition_broadcast(rec_sb[:], rec2[0:1, :])
                return rec_sb

            def norm_eager_j(j):
                # last pair: per-j chain so the proj tail isn't gated on a
                # whole-wave normalize (cuts ~4us off the tail)
                d1 = rec_pool.tile([1, W2], F32, tag="d1")
                nc.scalar.activation(out=d1[0:1, :],
                                     in_=avs[j][HD:HD + 1, :], func=AF.Copy)
                r1 = rec_pool.tile([1, W2], F32, tag="r1")
                nc.vector.reciprocal_approx_fast(out=r1[0:1, :], in_=d1[0:1, :])
                r1_sb = rec_pool.tile([128, W2], F32, tag="r1_sb")
                nc.gpsimd.partition_broadcast(r1_sb[:], r1[0:1, :])
                nc.vector.tensor_mul(out=dst[0:64, c0s[j]:c0s[j] + N],
                                     in0=avs[j][0:HD, 0:N],
                                     in1=r1_sb[0:64, 0:N])
                nc.vector.tensor_mul(out=dst[64:128, c0s[j]:c0s[j] + N],
                                     in0=avs[j][0:HD, N:W2],
                                     in1=r1_sb[64:128, N:W2])

            pending = [emit_scores(0), emit_scores(1)]
            # filler PE work (next pair's qk GEMMs / v GEMMs / proj) lands
            # HERE -- after this wave's first scores, so the PE chews on it
            # exactly while the scalar engine works through the exps the
            # first AVs depend on (was a ~1.5us PE stall per wave)
            filler = fillers[wave] if wave < len(fillers) else None
            if filler is not None:
                filler()
            # previous wave's first normalize half: frees av banks for this
            # wave's first AVs without stacking 8 multiplies on DVE at once
            if pending_norm:
                pending_norm.pop(0)()
            emit_av(0, pending[0])
            pending.append(emit_scores(2))
            if pending_norm:
                pending_norm.pop(0)()
            emit_av(1, pending[1])
            if eager:
                norm_eager_j(0)
                norm_eager_j(1)
            else:
                # first prep half here: its dcopies land on the scalar
                # queue BEFORE j3's exps, so the recip/broadcast chain
                # finishes inside this wave and the next wave's first AVs
                # find their banks already freeable
                r01 = norm_prep((0, 1))
                pending_norm.append(
                    lambda r=r01, f=norm_muls: f((0, 1), r))
            pending.append(emit_scores(3))
            emit_av(2, pending[2])
            emit_av(3, pending[3])
            if eager:
                norm_eager_j(2)
                norm_eager_j(3)
            else:
                r23 = norm_prep((2, 3))
                pending_norm.append(
                    lambda r=r23, f=norm_muls: f((2, 3), r))

    # ---------------- proj ----------------
    NRC = (R + 127) // 128  # 13 row chunks

    def emit_proj(rcs):
        for rc in rcs:
            r0 = 128 * rc
            nr = min(128, R - r0)
            ot = out_pool.tile([128, C], BF16, tag="out")
            for nh in range(2):
                ps = ps_pool.tile([128, 384], F32, tag="ps")
                for kc in range(KC):
                    nc.tensor.matmul(
                        out=ps[:nr, :],
                        lhsT=attn_outT[kc][:, r0:r0 + nr],
                        rhs=wp_all[:, kc, 384 * nh:384 * (nh + 1)],
                        start=(kc == 0), stop=(kc == KC - 1))
                nc.vector.tensor_add(out=ot[:nr, 384 * nh:384 * (nh + 1)],
                                     in0=ps[:nr, :],
                                     in1=bproj_bc[:nr, 384 * nh:384 * (nh + 1)])
            # one [*, 768] store per row chunk; alternate store queues so
            # descriptor issue parallelizes across the sync and gpsimd DGE
            eng = nc.sync if rc % 2 == 0 else nc.gpsimd
            eng.dma_start(out=out_d[r0:r0 + nr, :], in_=ot[:nr, :])

    # ---------------- emission schedule ----------------
    # ramp: q/k for pair 0 (kc-outer so compute starts while weight DMA
    # chunks still arrive), v for the first wave's batches
    emit_qk(0)
    emit_qk(6)
    emit_v(range(0, 4))

    def _qk(pn, ncols):
        return lambda: emit_qk_pair(pn, 6 + pn, ncols=ncols)

    # per pair p: wave-0 filler = p's own qk chunks 2,3 (needed by wave 1),
    # wave-1 filler = next pair's chunks 0,1 (needed by its wave 0).
    # pair 0's wave-0 filler is the remaining v GEMMs instead (its chunks
    # 2,3 came from the ramp).
    for p in range(NPAIR - 1):
        fa = (lambda: emit_v(range(4, 8))) if p == 0 else _qk(p, (2, 3))
        emit_attention_pair(p, fillers=(fa, _qk(p + 1, (0, 1))))

    # last pair: wave 0 (batches 0-3) with its chunks 2,3 as filler, then
    # wave 1 with the first proj row-chunks as filler, then the rest.
    # eager=True normalizes per-j so proj isn't gated on a wave-wide chain.
    emit_attention_pair(NPAIR - 1, waves=(0,), eager=True,
                        fillers=(_qk(NPAIR - 1, (2, 3)),))
    emit_attention_pair(NPAIR - 1, waves=(1,), eager=True,
                        fillers=(None, lambda: emit_proj(range(0, 6))))
    emit_proj(range(6, NRC))


_NC_CACHE = {}


def _get_nc():
    if "nc" not in _NC_CACHE:
        _NC_CACHE["nc"] = build_program()
    return _NC_CACHE["nc"]


def prep_aux(rpb_table, rel_idx):
    """Host-side prep: gather the bias from the two small aux inputs, lay it
    out per head PAIR in the kernel's transposed plane orientation
    [k_tok, q_tok*2] with zeroed CLS row/col, and exponentiate (bf16)."""
    import ml_dtypes
    bT = rpb_table[rel_idx.reshape(-1)].reshape(NP, NP, H)  # [q_idx, k_idx, h]
    bT = np.ascontiguousarray(bT.transpose(1, 0, 2))        # [k_idx, q_idx, h]
    bias0 = np.zeros((128, N, H), dtype=np.float32)
    bias0[1:128, 1:NP + 1, :] = bT[0:127]
    bias1 = np.zeros((128, N, H), dtype=np.float32)
    bias1[0:NP - 127, 1:NP + 1, :] = bT[127:NP]
    # partition-major [128, H, W2] so the device loads it as one DMA
    expb = np.zeros((128, H, W2), dtype=np.float32)
    for h in range(H):
        expb[:, h, 0:N] = np.exp(bias0[:, :, h])
        expb[:, h, N:W2] = np.exp(bias1[:, :, h])
    return expb.reshape(128, H * W2).astype(ml_dtypes.bfloat16)


def prep_weights(w_qkv, w_proj):
    """Host-side prep: transpose, fold the q scale into w_qkv, cast bf16."""
    import ml_dtypes
    wqkvT = np.array(w_qkv, dtype=np.float32).T.copy()
    wqkvT[:, 0:C] *= HD ** -0.5
    wpT = np.ascontiguousarray(np.asarray(w_proj, dtype=np.float32).T)
    return (wqkvT.astype(ml_dtypes.bfloat16), wpT.astype(ml_dtypes.bfloat16))


def make_in_maps(x, w_qkv, w_proj, b_proj, rpb_table, rel_idx):
    """Build the 8 per-core input maps (host prep: shard, transpose, bf16)."""
    import ml_dtypes
    x = np.asarray(x, dtype=np.float32)
    expb = prep_aux(
        np.asarray(rpb_table, dtype=np.float32), np.asarray(rel_idx).astype(np.int64))
    wqkvT, wpT = prep_weights(w_qkv, w_proj)
    bp = np.ascontiguousarray(np.asarray(b_proj, dtype=np.float32))
    xbf = x.astype(ml_dtypes.bfloat16)
    in_maps = []
    for c in range(NCORES):
        xT = np.ascontiguousarray(xbf[c * B:(c + 1) * B].reshape(R, C).T)
        in_maps.append({
            "xT": xT,
            "w_qkvT": wqkvT,
            "w_projT": wpT,
            "b_proj": bp,
            "expb": expb,
        })
    return in_maps


def kernel(x, w_qkv, w_proj, b_proj, rpb_table, rel_idx):
    from concourse.bass_utils import run_bass_kernel_spmd

    nc = _get_nc()
    in_maps = make_in_maps(x, w_qkv, w_proj, b_proj, rpb_table, rel_idx)
    res = run_bass_kernel_spmd(nc, in_maps, list(range(NCORES)))
    out = np.concatenate(
        [np.asarray(r["out"], dtype=np.float32).reshape(B, N, C)
         for r in res.results], axis=0)
    return out


# revision 33
# speedup vs baseline: 1.7535x; 1.0202x over previous
"""RPE (relative-position-bias) attention kernel for Trainium2, 8-core SPMD.

Full op (per reference):
  qkv = x @ w_qkv.T -> split q,k,v heads (H=12, hd=64), q *= hd**-0.5
  attn = q @ k.T ; attn[:, :, 1:, 1:] += rpb_table[rel_idx]  (per head)
  attn = softmax(attn, -1) ; out = (attn @ v) @ w_proj.T + b_proj

Sharding: data-parallel over batch. B=64 -> 8 batches per core. Weights
and bias-derived planes replicated to all cores. No collectives.

Per-core program (all matmuls bf16 operands, fp32 PSUM accumulation):
  - Inputs arrive bf16 and pre-transposed from the host: xT [768,1576],
    wqkvT [768,2304] (q columns pre-scaled by hd**-0.5), wpT [768,768].
  - qT,kT [768,1576] = w_chunk.T @ xT (transposed layout). v in natural
    layout [tokens, head, 65] with a ones column (softmax denominators
    fall out of the AV matmul for free).
  - The relative-position bias enters as exp(bias): probs = exp(s) *
    expb, expb planes host-precomputed bf16 in the transposed
    orientation [k_tok, q_tok] per head PAIR (two heads side by side,
    394 columns). exp runs on the scalar engine straight out of PSUM;
    the expb multiply runs on DVE in SBUF (bf16 fast path).
  - Heads are processed in pairs: score tiles [128,394] hold two heads.
  - Softmax normalization: denominators live in row 64 of the AV PSUM
    tile. Reciprocals run on DVE (keeps the scalar engine's activation
    table pinned to Exp -- the scalar-recip variant paid a 1.3us
    Exp<->Recip table reload per wave). The 4 reciprocal rows of a wave
    pack into one [1, 4*394] tile so a single gpsimd partition_broadcast
    per wave fans them out; the normalize multiplies run on DVE.
  - Engine balance: exp on scalar; qk PSUM->SBUF casts split between
    scalar (Copy shares the Exp table, no reload) and DVE; v casts on
    gpsimd; norm muls + expb muls + recips on DVE.
  - qk GEMMs for pair p+1 are emitted in two halves: column chunks 0,1
    between pair p's waves (needed by p+1 wave 0), chunks 2,3 after
    (needed only by p+1 wave 1). This keeps the PE fed across the wave
    boundary instead of bunching all 48 GEMMs at one point.
  - out = attn_outT.T @ wpT + b_proj  (fp32 output).
"""
import sys

sys.path.insert(0, '/opt/trn_rl_repo')

from contextlib import ExitStack

import numpy as np

import concourse.bass as bass
import concourse.bacc as bacc
import concourse.tile as tile
from concourse import mybir

# ---- problem dims (hardcoded per contract) ----
NCORES = 8
B_FULL = 64
B = B_FULL // NCORES     # 8 batches per core
N = 197                  # tokens (196 patches + CLS)
NP = 196
C = 768
H = 12
HD = 64
R = B * N                # 1576 rows per core
NPAIR = H // 2           # 6 head pairs
W2 = 2 * N               # 394 columns for a head pair

F32 = mybir.dt.float32
BF16 = mybir.dt.bfloat16
AF = mybir.ActivationFunctionType

import os
# CoreSim rejects reads of uninitialized PSUM; the exp deliberately reads a
# dead corner of the score tile (rows 69:128 of the chunk-1 columns, never
# consumed downstream). Sim runs memset it; hardware runs skip the cost.
SIM_SAFE = int(os.environ.get("KERNEL_SIM_SAFE", "0"))


def build_program():
    nc = bacc.Bacc("TRN2", target_bir_lowering=False, debug=False)

    x_d = nc.declare_dram_parameter("xT", [C, R], BF16, isOutput=False)
    wqkv_d = nc.declare_dram_parameter("w_qkvT", [C, 3 * C], BF16, isOutput=False)
    wp_d = nc.declare_dram_parameter("w_projT", [C, C], BF16, isOutput=False)
    bp_d = nc.declare_dram_parameter("b_proj", [C], F32, isOutput=False)
    # exp(bias) planes per head, transposed chunk-paired orientation:
    # expb [k_part 0:128, head, q 0:197 (k chunk 0) ++ q 0:197 (k chunk 1)]
    # (chunk 1 rows beyond k=196 are 1.0 and multiply unused garbage);
    # partition-major host layout so it loads as ONE contiguous DMA
    expb_d = nc.declare_dram_parameter("expb", [128, H * W2], BF16,
                                       isOutput=False)
    out_d = nc.declare_dram_parameter("out", [R, C], BF16, isOutput=True)

    with tile.TileContext(nc) as tc:
        with ExitStack() as ctx:
            _emit(ctx, tc, nc, x_d, wqkv_d, wp_d, bp_d, expb_d, out_d)
    nc.compile()
    return nc


def _emit(ctx, tc, nc, x_d, wqkv_d, wp_d, bp_d, expb_d, out_d):
    singles = ctx.enter_context(tc.tile_pool(name="singles", bufs=1))
    ps_pool = ctx.enter_context(tc.tile_pool(name="ps", bufs=4, space="PSUM"))
    av_ps = ctx.enter_context(tc.tile_pool(name="av_ps", bufs=4, space="PSUM"))
    probs_pool = ctx.enter_context(tc.tile_pool(name="probs", bufs=8))
    rec_pool = ctx.enter_context(tc.tile_pool(name="rec", bufs=2))
    out_pool = ctx.enter_context(tc.tile_pool(name="outp", bufs=4))

    KC = C // 128  # 6 contraction chunks

    # ---------------- load operands (already bf16 + transposed) ----------
    xT = []     # 6 x [128, R] bf16
    wqkvT = []  # 6 x [128, 1536] bf16 (q,k columns; v loads separately)
    for kc in range(KC):
        t = singles.tile([128, R], BF16, tag=f"xT{kc}", name=f"xT{kc}")
        # split the x chunks across the gpsimd and scalar DGE queues so
        # the later contraction chunks aren't serialized behind the first
        # ones' descriptor issue (the ramp's qk GEMMs consume them at
        # ~1.6us/chunk, about the single-queue delivery rate)
        eng = nc.gpsimd if kc % 2 == 0 else nc.scalar
        eng.dma_start(out=t[:], in_=x_d[128 * kc:128 * (kc + 1), :])
        xT.append(t)
        w = singles.tile([128, 2 * C], BF16, tag=f"wqkvT{kc}", name=f"wqkvT{kc}")
        nc.sync.dma_start(out=w[:],
                          in_=wqkv_d[128 * kc:128 * (kc + 1), 0:2 * C])
        wqkvT.append(w)
    # remaining inputs issue from the (idle at startup) scalar DGE queue so the xT/wqkv chunk DMAs aren't stuck behind them.
    # v columns of w_qkv: one strided DMA filling all six chunk tiles'
    # v-column ranges (6 descriptors/partition instead of 6 DMA issues)
    vcols = singles.tile([128, KC, C], BF16, tag="wqkv_v", name="wqkv_v")
    nc.scalar.dma_start(
        out=vcols[:],
        in_=bass.AP(tensor=wqkv_d, offset=2 * C,
                    ap=[[3 * C, 128], [128 * 3 * C, KC], [1, C]]))
    # wp: one strided DMA for all 6 contraction chunks
    wp_all = singles.tile([128, KC, C], BF16, tag="wp_all", name="wp_all")
    nc.scalar.dma_start(
        out=wp_all[:],
        in_=bass.AP(tensor=wp_d, offset=0,
                    ap=[[C, 128], [128 * C, KC], [1, C]]))

    bproj_bc = singles.tile([128, C], F32, tag="bproj")
    nc.scalar.dma_start(out=bproj_bc[:],
                        in_=bass.AP(tensor=bp_d, offset=0, ap=[[0, 128], [1, C]]))

    expb_all = singles.tile([128, H, W2], BF16, tag="expb", name="expb")
    nc.scalar.dma_start(out=expb_all[:], in_=expb_d[:, :])
    expb = [expb_all[:, h, :] for h in range(H)]  # [128, 394] per head

    # ---------------- QKV ----------------
    NCHUNK = 4
    NW = R // NCHUNK  # 394 columns per psum tile

    # qk_sb[ft][ncol]: SEPARATE [128, 394] tiles per column chunk (2
    # batches each). Dependency tracking is tile-granular for matmul
    # operands, so per-chunk tiles let a wave's scores wait on exactly the
    # one copy they need instead of all four.
    qk_sb = [[None] * NCHUNK for _ in range(12)]

    def emit_qk(ft):
        # kc-outer, all four column tiles held across the contraction: the
        # first chunks compute while later weight DMA chunks still arrive
        # (only matters for the ramp pair ft 0/6)
        tiles = [ps_pool.tile([128, NW], F32, tag="ps", name=f"qkps{ncol}")
                 for ncol in range(NCHUNK)]
        for kc in range(KC):
            for ncol in range(NCHUNK):
                nc.tensor.matmul(
                    out=tiles[ncol][:],
                    lhsT=wqkvT[kc][:, 128 * ft:128 * (ft + 1)],
                    rhs=xT[kc][:, NW * ncol:NW * (ncol + 1)],
                    start=(kc == 0), stop=(kc == KC - 1))
        for ncol in range(NCHUNK):
            tg = (ft % 6) % 3 + (3 if ft >= 6 else 0)
            dst = singles.tile([128, NW], BF16, tag=f"qk{tg}_{ncol}",
                               name=f"qk{ft}_{ncol}")
            qk_sb[ft][ncol] = dst
            nc.vector.tensor_copy(out=dst[:], in_=tiles[ncol][:])

    def emit_qk_pair(fta, ftb, ncols):
        # q/k tile pair, GEMMs + copies interleaved per column chunk.
        # Copies run as scalar-Copy (Copy shares the Exp act table, so no
        # table reload) -- DVE carries the recip/norm/expb work instead.
        for ncol in ncols:
            for i, ft in enumerate((fta, ftb)):
                ps = ps_pool.tile([128, NW], F32, tag="ps", name="qkps")
                for kc in range(KC):
                    nc.tensor.matmul(
                        out=ps[:],
                        lhsT=wqkvT[kc][:, 128 * ft:128 * (ft + 1)],
                        rhs=xT[kc][:, NW * ncol:NW * (ncol + 1)],
                        start=(kc == 0), stop=(kc == KC - 1))
                tg = (ft % 6) % 3 + (3 if ft >= 6 else 0)
                dst = singles.tile([128, NW], BF16, tag=f"qk{tg}_{ncol}",
                                   name=f"qk{ft}_{ncol}")
                qk_sb[ft][ncol] = dst
                if (ncol + i) % 2 == 0:
                    nc.scalar.activation(out=dst[:], in_=ps[:], func=AF.Copy)
                else:
                    nc.vector.tensor_copy(out=dst[:], in_=ps[:])

    # v_aug[b][c]: [128, 12, 65] bf16 (col 64 = ones)
    v_aug = [[None, None] for _ in range(B)]

    def emit_v(batches):
        for b in batches:
            for cchunk, (r0, nr) in enumerate(((N * b, 128), (N * b + 128, N - 128))):
                dst = singles.tile([128, H, HD + 1], BF16, tag=f"v{b}_{cchunk}",
                                   name=f"v{b}_{cchunk}")
                v_aug[b][cchunk] = dst
                nc.vector.memset(dst[:, :, HD:HD + 1], 1.0)
                for nh in range(2):
                    ps = ps_pool.tile([128, 384], F32, tag="ps")
                    for kc in range(KC):
                        nc.tensor.matmul(
                            out=ps[:nr, :],
                            lhsT=xT[kc][:, r0:r0 + nr],
                            rhs=vcols[:, kc, 384 * nh:384 * (nh + 1)],
                            start=(kc == 0), stop=(kc == KC - 1))
                    # PSUM->SBUF casts split DVE / scalar-Copy (gpsimd
                    # cannot read PSUM); both engines are idle at this
                    # point in the schedule
                    if (b + cchunk + nh) % 2 == 0:
                        nc.vector.tensor_copy(
                            out=dst[:nr, 6 * nh:6 * (nh + 1), 0:HD],
                            in_=ps[:nr, :].rearrange("p (h d) -> p h d", h=6))
                    else:
                        nc.scalar.activation(
                            out=dst[:nr, 6 * nh:6 * (nh + 1), 0:HD],
                            in_=ps[:nr, :].rearrange("p (h d) -> p h d", h=6),
                            func=AF.Copy)

    # deferred normalize-multiply closures (two half-flushes per wave)
    pending_norm = []

    def flush_norm():
        while pending_norm:
            pending_norm.pop(0)()

    # attn output, transposed: 6 tiles [128, R] bf16 (pair p = heads 2p,2p+1)
    attn_outT = []
    for p in range(NPAIR):
        attn_outT.append(singles.tile([128, R], BF16, tag=f"aoT{p}",
                                      name=f"aoT{p}"))

    def emit_attention_pair(p, waves=(0, 1), fillers=(None, None),
                            eager=False):
        N1 = N - 128  # 69
        dst = attn_outT[p]
        for wave in waves:
            avs = []
            c0s = []

            def emit_scores(j):
                b = 4 * wave + j
                c0s.append(N * b)
                qt = qk_sb[p][b // 2]
                kt = qk_sb[6 + p][b // 2]
                o = N * (b % 2)
                qh = [qt[0:64, o:o + N], qt[64:128, o:o + N]]
                kh = [kt[0:64, o:o + N], kt[64:128, o:o + N]]
                phs = []
                for hh in range(2):
                    sth = ps_pool.tile([128, W2], F32, tag="ps",
                                       name=f"sth{hh}")
                    if SIM_SAFE:
                        nc.vector.memset(sth[64:128, N:W2], 0.0)
                    nc.tensor.matmul(out=sth[:, 0:N],
                                     lhsT=kh[hh][:, 0:128], rhs=qh[hh],
                                     start=True, stop=True)
                    nc.tensor.matmul(out=sth[0:N1, N:W2],
                                     lhsT=kh[hh][:, 128:N], rhs=qh[hh],
                                     start=True, stop=True)
                    ph = probs_pool.tile([128, W2], BF16, tag="probs")
                    nc.scalar.activation(out=ph[:], in_=sth[:], func=AF.Exp)
                    # expb multiply: all-SBUF bf16 fast DVE path, kept on
                    # DVE only -- on gpsimd it sat behind the broadcast in
                    # the in-order queue and stalled the AV matmuls
                    nc.vector.tensor_mul(out=ph[:], in0=ph[:],
                                         in1=expb[2 * p + hh])
                    phs.append(ph)
                return phs

            def emit_av(j, phs):
                b = 4 * wave + j
                av = av_ps.tile([HD + 1, W2], F32, tag="av")
                avs.append(av)
                for hh in range(2):
                    h = 2 * p + hh
                    nc.tensor.matmul(out=av[:, N * hh:N * hh + N],
                                     lhsT=v_aug[b][0][:, h, :],
                                     rhs=phs[hh][:, 0:N],
                                     start=True, stop=False)
                    nc.tensor.matmul(out=av[:, N * hh:N * hh + N],
                                     lhsT=v_aug[b][1][0:N1, h, :],
                                     rhs=phs[hh][0:N1, N:W2],
                                     start=False, stop=True)

            def norm_muls(js, rec_sb, avs=avs, c0s=c0s):
                for j in js:
                    o = W2 * (j % 2)
                    nc.vector.tensor_mul(out=dst[0:64, c0s[j]:c0s[j] + N],
                                         in0=avs[j][0:HD, 0:N],
                                         in1=rec_sb[0:64, o:o + N])
                    nc.vector.tensor_mul(out=dst[64:128, c0s[j]:c0s[j] + N],
                                         in0=avs[j][0:HD, N:W2],
                                         in1=rec_sb[64:128, o + N:o + W2])

            def norm_prep(js):
                # softmax denominators for two j's: copy the PSUM rows to
                # SBUF on the scalar engine (Copy shares the Exp table -> no
                # reload; the custom-DVE approx reciprocal reads garbage
                # from PSUM), then a packed ~18-bit reciprocal_approx_fast
                # on DVE and a gpsimd partition_broadcast. Emitted in two
                # halves mid-wave so the chain finishes before the next
                # wave needs the av banks; only the multiplies are deferred.
                d2 = rec_pool.tile([1, 2 * W2], F32, tag="d2")
                for i, j in enumerate(js):
                    nc.scalar.activation(out=d2[0:1, W2 * i:W2 * (i + 1)],
                                         in_=avs[j][HD:HD + 1, :],
                                         func=AF.Copy)
                rec2 = rec_pool.tile([1, 2 * W2], F32, tag="rec2")
                nc.vector.reciprocal_approx_fast(out=rec2[0:1, :],
                                                 in_=d2[0:1, :])
                rec_sb = rec_pool.tile([128, 2 * W2], F32, tag="rec_sb2")
                nc.gpsimd.partition_broadcast(rec_sb[:], rec2[0:1, :])
                return rec_sb

            def norm_eager_j(j):
                # last pair: per-j chain so the proj tail isn't gated on a
                # whole-wave normalize (cuts ~4us off the tail)
                d1 = rec_pool.tile([1, W2], F32, tag="d1")
                nc.scalar.activation(out=d1[0:1, :],
                                     in_=avs[j][HD:HD + 1, :], func=AF.Copy)
                r1 = rec_pool.tile([1, W2], F32, tag="r1")
                nc.vector.reciprocal_approx_fast(out=r1[0:1, :], in_=d1[0:1, :])
                r1_sb = rec_pool.tile([128, W2], F32, tag="r1_sb")
                nc.gpsimd.partition_broadcast(r1_sb[:], r1[0:1, :])
                nc.vector.tensor_mul(out=dst[0:64, c0s[j]:c0s[j] + N],
                                     in0=avs[j][0:HD, 0:N],
                                     in1=r1_sb[0:64, 0:N])
                nc.vector.tensor_mul(out=dst[64:128, c0s[j]:c0s[j] + N],
                                     in0=avs[j][0:HD, N:W2],
                                     in1=r1_sb[64:128, N:W2])

            pending = [emit_scores(0), emit_scores(1)]
            # filler PE work (next pair's qk GEMMs / v GEMMs / proj) lands
            # HERE -- after this wave's first scores, so the PE chews on it
            # exactly while the scalar engine works through the exps the
            # first AVs depend on (was a ~1.5us PE stall per wave)
            filler = fillers[wave] if wave < len(fillers) else None
            if filler is not None:
                filler()
            # previous wave's first normalize half: frees av banks for this
            # wave's first AVs without stacking 8 multiplies on DVE at once
            if pending_norm:
                pending_norm.pop(0)()
            emit_av(0, pending[0])
            pending.append(emit_scores(2))
            if pending_norm:
                pending_norm.pop(0)()
            emit_av(1, pending[1])
            if eager:
                norm_eager_j(0)
                norm_eager_j(1)
            else:
                # first prep half here: its dcopies land on the scalar
                # queue BEFORE j3's exps, so the recip/broadcast chain
                # finishes inside this wave and the next wave's first AVs
                # find their banks already freeable
                r01 = norm_prep((0, 1))
                pending_norm.append(
                    lambda r=r01, f=norm_muls: f((0, 1), r))
            pending.append(emit_scores(3))
            emit_av(2, pending[2])
            emit_av(3, pending[3])
            if eager:
                norm_eager_j(2)
                norm_eager_j(3)
            else:
                r23 = norm_prep((2, 3))
                pending_norm.append(
                    lambda r=r23, f=norm_muls: f((2, 3), r))

    # ---------------- proj ----------------
    NRC = (R + 127) // 128  # 13 row chunks

    def emit_proj(rcs):
        for rc in rcs:
            r0 = 128 * rc
            nr = min(128, R - r0)
            ot = out_pool.tile([128, C], BF16, tag="out")
            for nh in range(2):
                ps = ps_pool.tile([128, 384], F32, tag="ps")
                for kc in range(KC):
                    nc.tensor.matmul(
                        out=ps[:nr, :],
                        lhsT=attn_outT[kc][:, r0:r0 + nr],
                        rhs=wp_all[:, kc, 384 * nh:384 * (nh + 1)],
                        start=(kc == 0), stop=(kc == KC - 1))
                nc.vector.tensor_add(out=ot[:nr, 384 * nh:384 * (nh + 1)],
                                     in0=ps[:nr, :],
                                     in1=bproj_bc[:nr, 384 * nh:384 * (nh + 1)])
            # one [*, 768] store per row chunk; alternate store queues so
            # descriptor issue parallelizes across the sync and gpsimd DGE
            eng = nc.sync if rc % 2 == 0 else nc.gpsimd
            eng.dma_start(out=out_d[r0:r0 + nr, :], in_=ot[:nr, :])

    # ---------------- emission schedule ----------------
    # ramp: q/k for pair 0 (kc-outer so compute starts while weight DMA
    # chunks still arrive), v for the first wave's batches
    emit_qk(0)
    emit_qk(6)
    emit_v(range(0, 4))

    def _qk(pn, ncols):
        return lambda: emit_qk_pair(pn, 6 + pn, ncols=ncols)

    # per pair p: wave-0 filler = p's own qk chunks 2,3 (needed by wave 1),
    # wave-1 filler = next pair's chunks 0,1 (needed by its wave 0).
    # pair 0's wave-0 filler is the remaining v GEMMs instead (its chunks
    # 2,3 came from the ramp).
    for p in range(NPAIR - 1):
        fa = (lambda: emit_v(range(4, 8))) if p == 0 else _qk(p, (2, 3))
        emit_attention_pair(p, fillers=(fa, _qk(p + 1, (0, 1))))

    # last pair: wave 0 (batches 0-3) with its chunks 2,3 as filler, then
    # wave 1 with the first proj row-chunks as filler, then the rest.
    # eager=True normalizes per-j so proj isn't gated on a wave-wide chain.
    emit_attention_pair(NPAIR - 1, waves=(0,), eager=True,
                        fillers=(_qk(NPAIR - 1, (2, 3)),))
    emit_attention_pair(NPAIR - 1, waves=(1,), eager=True,
                        fillers=(None, lambda: emit_proj(range(0, 6))))
    emit_proj(range(6, NRC))


_NC_CACHE = {}


def _get_nc():
    if "nc" not in _NC_CACHE:
        _NC_CACHE["nc"] = build_program()
    return _NC_CACHE["nc"]


def prep_aux(rpb_table, rel_idx):
    """Host-side prep: gather the bias from the two small aux inputs, lay it
    out per head PAIR in the kernel's transposed plane orientation
    [k_tok, q_tok*2] with zeroed CLS row/col, and exponentiate (bf16)."""
    import ml_dtypes
    bT = rpb_table[rel_idx.reshape(-1)].reshape(NP, NP, H)  # [q_idx, k_idx, h]
    bT = np.ascontiguousarray(bT.transpose(1, 0, 2))        # [k_idx, q_idx, h]
    bias0 = np.zeros((128, N, H), dtype=np.float32)
    bias0[1:128, 1:NP + 1, :] = bT[0:127]
    bias1 = np.zeros((128, N, H), dtype=np.float32)
    bias1[0:NP - 127, 1:NP + 1, :] = bT[127:NP]
    # partition-major [128, H, W2] so the device loads it as one DMA
    expb = np.zeros((128, H, W2), dtype=np.float32)
    for h in range(H):
        expb[:, h, 0:N] = np.exp(bias0[:, :, h])
        expb[:, h, N:W2] = np.exp(bias1[:, :, h])
    return expb.reshape(128, H * W2).astype(ml_dtypes.bfloat16)


def prep_weights(w_qkv, w_proj):
    """Host-side prep: transpose, fold the q scale into w_qkv, cast bf16."""
    import ml_dtypes
    wqkvT = np.array(w_qkv, dtype=np.float32).T.copy()
    wqkvT[:, 0:C] *= HD ** -0.5
    wpT = np.ascontiguousarray(np.asarray(w_proj, dtype=np.float32).T)
    return (wqkvT.astype(ml_dtypes.bfloat16), wpT.astype(ml_dtypes.bfloat16))


def make_in_maps(x, w_qkv, w_proj, b_proj, rpb_table, rel_idx):
    """Build the 8 per-core input maps (host prep: shard, transpose, bf16)."""
    import ml_dtypes
    x = np.asarray(x, dtype=np.float32)
    expb = prep_aux(
        np.asarray(rpb_table, dtype=np.float32), np.asarray(rel_idx).astype(np.int64))
    wqkvT, wpT = prep_weights(w_qkv, w_proj)
    bp = np.ascontiguousarray(np.asarray(b_proj, dtype=np.float32))
    xbf = x.astype(ml_dtypes.bfloat16)
    in_maps = []
    for c in range(NCORES):
        xT = np.ascontiguousarray(xbf[c * B:(c + 1) * B].reshape(R, C).T)
        in_maps.append({
            "xT": xT,
            "w_qkvT": wqkvT,
            "w_projT": wpT,
            "b_proj": bp,
            "expb": expb,
        })
    return in_maps


def kernel(x, w_qkv, w_proj, b_proj, rpb_table, rel_idx):
    from concourse.bass_utils import run_bass_kernel_spmd

    nc = _get_nc()
    in_maps = make_in_maps(x, w_qkv, w_proj, b_proj, rpb_table, rel_idx)
    res = run_bass_kernel_spmd(nc, in_maps, list(range(NCORES)))
    out = np.concatenate(
        [np.asarray(r["out"], dtype=np.float32).reshape(B, N, C)
         for r in res.results], axis=0)
    return out


# revision 34
# speedup vs baseline: 1.7651x; 1.0066x over previous
"""RPE (relative-position-bias) attention kernel for Trainium2, 8-core SPMD.

Full op (per reference):
  qkv = x @ w_qkv.T -> split q,k,v heads (H=12, hd=64), q *= hd**-0.5
  attn = q @ k.T ; attn[:, :, 1:, 1:] += rpb_table[rel_idx]  (per head)
  attn = softmax(attn, -1) ; out = (attn @ v) @ w_proj.T + b_proj

Sharding: data-parallel over batch. B=64 -> 8 batches per core. Weights
and bias-derived planes replicated to all cores. No collectives.

Per-core program (all matmuls bf16 operands, fp32 PSUM accumulation):
  - Inputs arrive bf16 and pre-transposed from the host: xT [768,1576],
    wqkvT [768,2304] (q columns pre-scaled by hd**-0.5), wpT [768,768].
  - qT,kT [768,1576] = w_chunk.T @ xT (transposed layout). v in natural
    layout [tokens, head, 65] with a ones column (softmax denominators
    fall out of the AV matmul for free).
  - The relative-position bias enters as exp(bias): probs = exp(s) *
    expb, expb planes host-precomputed bf16 in the transposed
    orientation [k_tok, q_tok] per head PAIR (two heads side by side,
    394 columns). exp runs on the scalar engine straight out of PSUM;
    the expb multiply runs on DVE in SBUF (bf16 fast path).
  - Heads are processed in pairs: score tiles [128,394] hold two heads.
  - Softmax normalization: denominators live in row 64 of the AV PSUM
    tile. Reciprocals run on DVE (keeps the scalar engine's activation
    table pinned to Exp -- the scalar-recip variant paid a 1.3us
    Exp<->Recip table reload per wave). The 4 reciprocal rows of a wave
    pack into one [1, 4*394] tile so a single gpsimd partition_broadcast
    per wave fans them out; the normalize multiplies run on DVE.
  - Engine balance: exp on scalar; qk PSUM->SBUF casts split between
    scalar (Copy shares the Exp table, no reload) and DVE; v casts on
    gpsimd; norm muls + expb muls + recips on DVE.
  - qk GEMMs for pair p+1 are emitted in two halves: column chunks 0,1
    between pair p's waves (needed by p+1 wave 0), chunks 2,3 after
    (needed only by p+1 wave 1). This keeps the PE fed across the wave
    boundary instead of bunching all 48 GEMMs at one point.
  - out = attn_outT.T @ wpT + b_proj  (fp32 output).
"""
import sys

sys.path.insert(0, '/opt/trn_rl_repo')

from contextlib import ExitStack

import numpy as np

import concourse.bass as bass
import concourse.bacc as bacc
import concourse.tile as tile
from concourse import mybir

# ---- problem dims (hardcoded per contract) ----
NCORES = 8
B_FULL = 64
B = B_FULL // NCORES     # 8 batches per core
N = 197                  # tokens (196 patches + CLS)
NP = 196
C = 768
H = 12
HD = 64
R = B * N                # 1576 rows per core
NPAIR = H // 2           # 6 head pairs
W2 = 2 * N               # 394 columns for a head pair

F32 = mybir.dt.float32
BF16 = mybir.dt.bfloat16
AF = mybir.ActivationFunctionType

import os
# CoreSim rejects reads of uninitialized PSUM; the exp deliberately reads a
# dead corner of the score tile (rows 69:128 of the chunk-1 columns, never
# consumed downstream). Sim runs memset it; hardware runs skip the cost.
SIM_SAFE = int(os.environ.get("KERNEL_SIM_SAFE", "0"))


def build_program():
    nc = bacc.Bacc("TRN2", target_bir_lowering=False, debug=False)

    x_d = nc.declare_dram_parameter("xT", [C, R], BF16, isOutput=False)
    wqkv_d = nc.declare_dram_parameter("w_qkvT", [C, 3 * C], BF16, isOutput=False)
    wp_d = nc.declare_dram_parameter("w_projT", [C, C], BF16, isOutput=False)
    bp_d = nc.declare_dram_parameter("b_proj", [C], F32, isOutput=False)
    # exp(bias) planes per head, transposed chunk-paired orientation:
    # expb [k_part 0:128, head, q 0:197 (k chunk 0) ++ q 0:197 (k chunk 1)]
    # (chunk 1 rows beyond k=196 are 1.0 and multiply unused garbage);
    # partition-major host layout so it loads as ONE contiguous DMA
    expb_d = nc.declare_dram_parameter("expb", [128, H * W2], BF16,
                                       isOutput=False)
    out_d = nc.declare_dram_parameter("out", [R, C], BF16, isOutput=True)

    with tile.TileContext(nc) as tc:
        with ExitStack() as ctx:
            _emit(ctx, tc, nc, x_d, wqkv_d, wp_d, bp_d, expb_d, out_d)
    nc.compile()
    return nc


def _emit(ctx, tc, nc, x_d, wqkv_d, wp_d, bp_d, expb_d, out_d):
    singles = ctx.enter_context(tc.tile_pool(name="singles", bufs=1))
    ps_pool = ctx.enter_context(tc.tile_pool(name="ps", bufs=4, space="PSUM"))
    av_ps = ctx.enter_context(tc.tile_pool(name="av_ps", bufs=4, space="PSUM"))
    probs_pool = ctx.enter_context(tc.tile_pool(name="probs", bufs=12))
    rec_pool = ctx.enter_context(tc.tile_pool(name="rec", bufs=2))
    out_pool = ctx.enter_context(tc.tile_pool(name="outp", bufs=4))

    KC = C // 128  # 6 contraction chunks

    # ---------------- load operands (already bf16 + transposed) ----------
    xT = []     # 6 x [128, R] bf16
    wqkvT = []  # 6 x [128, 1536] bf16 (q,k columns; v loads separately)
    for kc in range(KC):
        t = singles.tile([128, R], BF16, tag=f"xT{kc}", name=f"xT{kc}")
        # split the x chunks across the gpsimd and scalar DGE queues so
        # the later contraction chunks aren't serialized behind the first
        # ones' descriptor issue (the ramp's qk GEMMs consume them at
        # ~1.6us/chunk, about the single-queue delivery rate)
        eng = nc.gpsimd if kc % 2 == 0 else nc.scalar
        eng.dma_start(out=t[:], in_=x_d[128 * kc:128 * (kc + 1), :])
        xT.append(t)
        w = singles.tile([128, 2 * C], BF16, tag=f"wqkvT{kc}", name=f"wqkvT{kc}")
        nc.sync.dma_start(out=w[:],
                          in_=wqkv_d[128 * kc:128 * (kc + 1), 0:2 * C])
        wqkvT.append(w)
    # remaining inputs issue from the (idle at startup) scalar DGE queue so the xT/wqkv chunk DMAs aren't stuck behind them.
    # v columns of w_qkv: one strided DMA filling all six chunk tiles'
    # v-column ranges (6 descriptors/partition instead of 6 DMA issues)
    vcols = singles.tile([128, KC, C], BF16, tag="wqkv_v", name="wqkv_v")
    nc.scalar.dma_start(
        out=vcols[:],
        in_=bass.AP(tensor=wqkv_d, offset=2 * C,
                    ap=[[3 * C, 128], [128 * 3 * C, KC], [1, C]]))
    # wp: one strided DMA for all 6 contraction chunks
    wp_all = singles.tile([128, KC, C], BF16, tag="wp_all", name="wp_all")
    nc.scalar.dma_start(
        out=wp_all[:],
        in_=bass.AP(tensor=wp_d, offset=0,
                    ap=[[C, 128], [128 * C, KC], [1, C]]))

    bproj_bc = singles.tile([128, C], F32, tag="bproj")
    nc.scalar.dma_start(out=bproj_bc[:],
                        in_=bass.AP(tensor=bp_d, offset=0, ap=[[0, 128], [1, C]]))

    expb_all = singles.tile([128, H, W2], BF16, tag="expb", name="expb")
    nc.scalar.dma_start(out=expb_all[:], in_=expb_d[:, :])
    expb = [expb_all[:, h, :] for h in range(H)]  # [128, 394] per head

    # ---------------- QKV ----------------
    NCHUNK = 4
    NW = R // NCHUNK  # 394 columns per psum tile

    # qk_sb[ft][ncol]: SEPARATE [128, 394] tiles per column chunk (2
    # batches each). Dependency tracking is tile-granular for matmul
    # operands, so per-chunk tiles let a wave's scores wait on exactly the
    # one copy they need instead of all four.
    qk_sb = [[None] * NCHUNK for _ in range(12)]

    def emit_qk(ft):
        # kc-outer, all four column tiles held across the contraction: the
        # first chunks compute while later weight DMA chunks still arrive
        # (only matters for the ramp pair ft 0/6)
        tiles = [ps_pool.tile([128, NW], F32, tag="ps", name=f"qkps{ncol}")
                 for ncol in range(NCHUNK)]
        for kc in range(KC):
            for ncol in range(NCHUNK):
                nc.tensor.matmul(
                    out=tiles[ncol][:],
                    lhsT=wqkvT[kc][:, 128 * ft:128 * (ft + 1)],
                    rhs=xT[kc][:, NW * ncol:NW * (ncol + 1)],
                    start=(kc == 0), stop=(kc == KC - 1))
        for ncol in range(NCHUNK):
            tg = (ft % 6) % 3 + (3 if ft >= 6 else 0)
            dst = singles.tile([128, NW], BF16, tag=f"qk{tg}_{ncol}",
                               name=f"qk{ft}_{ncol}")
            qk_sb[ft][ncol] = dst
            nc.vector.tensor_copy(out=dst[:], in_=tiles[ncol][:])

    def emit_qk_pair(fta, ftb, ncols):
        # q/k tile pair, GEMMs + copies interleaved per column chunk.
        # Copies run as scalar-Copy (Copy shares the Exp act table, so no
        # table reload) -- DVE carries the recip/norm/expb work instead.
        for ncol in ncols:
            for i, ft in enumerate((fta, ftb)):
                ps = ps_pool.tile([128, NW], F32, tag="ps", name="qkps")
                for kc in range(KC):
                    nc.tensor.matmul(
                        out=ps[:],
                        lhsT=wqkvT[kc][:, 128 * ft:128 * (ft + 1)],
                        rhs=xT[kc][:, NW * ncol:NW * (ncol + 1)],
                        start=(kc == 0), stop=(kc == KC - 1))
                tg = (ft % 6) % 3 + (3 if ft >= 6 else 0)
                dst = singles.tile([128, NW], BF16, tag=f"qk{tg}_{ncol}",
                                   name=f"qk{ft}_{ncol}")
                qk_sb[ft][ncol] = dst
                if (ncol + i) % 2 == 0:
                    nc.scalar.activation(out=dst[:], in_=ps[:], func=AF.Copy)
                else:
                    nc.vector.tensor_copy(out=dst[:], in_=ps[:])

    # v_aug[b][c]: [128, 12, 65] bf16 (col 64 = ones)
    v_aug = [[None, None] for _ in range(B)]

    def emit_v(batches):
        for b in batches:
            for cchunk, (r0, nr) in enumerate(((N * b, 128), (N * b + 128, N - 128))):
                dst = singles.tile([128, H, HD + 1], BF16, tag=f"v{b}_{cchunk}",
                                   name=f"v{b}_{cchunk}")
                v_aug[b][cchunk] = dst
                nc.vector.memset(dst[:, :, HD:HD + 1], 1.0)
                for nh in range(2):
                    ps = ps_pool.tile([128, 384], F32, tag="ps")
                    for kc in range(KC):
                        nc.tensor.matmul(
                            out=ps[:nr, :],
                            lhsT=xT[kc][:, r0:r0 + nr],
                            rhs=vcols[:, kc, 384 * nh:384 * (nh + 1)],
                            start=(kc == 0), stop=(kc == KC - 1))
                    # PSUM->SBUF casts split DVE / scalar-Copy (gpsimd
                    # cannot read PSUM); both engines are idle at this
                    # point in the schedule
                    if (b + cchunk + nh) % 2 == 0:
                        nc.vector.tensor_copy(
                            out=dst[:nr, 6 * nh:6 * (nh + 1), 0:HD],
                            in_=ps[:nr, :].rearrange("p (h d) -> p h d", h=6))
                    else:
                        nc.scalar.activation(
                            out=dst[:nr, 6 * nh:6 * (nh + 1), 0:HD],
                            in_=ps[:nr, :].rearrange("p (h d) -> p h d", h=6),
                            func=AF.Copy)

    # deferred normalize-multiply closures (two half-flushes per wave)
    pending_norm = []

    def flush_norm():
        while pending_norm:
            pending_norm.pop(0)()

    # attn output, transposed: 6 tiles [128, R] bf16 (pair p = heads 2p,2p+1)
    attn_outT = []
    for p in range(NPAIR):
        attn_outT.append(singles.tile([128, R], BF16, tag=f"aoT{p}",
                                      name=f"aoT{p}"))

    def emit_attention_pair(p, waves=(0, 1), fillers=(None, None),
                            eager=False):
        N1 = N - 128  # 69
        dst = attn_outT[p]
        for wave in waves:
            avs = []
            c0s = []

            def emit_scores(j):
                b = 4 * wave + j
                c0s.append(N * b)
                qt = qk_sb[p][b // 2]
                kt = qk_sb[6 + p][b // 2]
                o = N * (b % 2)
                qh = [qt[0:64, o:o + N], qt[64:128, o:o + N]]
                kh = [kt[0:64, o:o + N], kt[64:128, o:o + N]]
                phs = []
                for hh in range(2):
                    sth = ps_pool.tile([128, W2], F32, tag="ps",
                                       name=f"sth{hh}")
                    if SIM_SAFE:
                        nc.vector.memset(sth[64:128, N:W2], 0.0)
                    nc.tensor.matmul(out=sth[:, 0:N],
                                     lhsT=kh[hh][:, 0:128], rhs=qh[hh],
                                     start=True, stop=True)
                    nc.tensor.matmul(out=sth[0:N1, N:W2],
                                     lhsT=kh[hh][:, 128:N], rhs=qh[hh],
                                     start=True, stop=True)
                    ph = probs_pool.tile([128, W2], BF16, tag="probs")
                    nc.scalar.activation(out=ph[:], in_=sth[:], func=AF.Exp)
                    # expb multiply: all-SBUF bf16 fast DVE path, kept on
                    # DVE only -- on gpsimd it sat behind the broadcast in
                    # the in-order queue and stalled the AV matmuls
                    nc.vector.tensor_mul(out=ph[:], in0=ph[:],
                                         in1=expb[2 * p + hh])
                    phs.append(ph)
                return phs

            def emit_av(j, phs):
                b = 4 * wave + j
                av = av_ps.tile([HD + 1, W2], F32, tag="av")
                avs.append(av)
                for hh in range(2):
                    h = 2 * p + hh
                    nc.tensor.matmul(out=av[:, N * hh:N * hh + N],
                                     lhsT=v_aug[b][0][:, h, :],
                                     rhs=phs[hh][:, 0:N],
                                     start=True, stop=False)
                    nc.tensor.matmul(out=av[:, N * hh:N * hh + N],
                                     lhsT=v_aug[b][1][0:N1, h, :],
                                     rhs=phs[hh][0:N1, N:W2],
                                     start=False, stop=True)

            def norm_muls(js, rec_sb, avs=avs, c0s=c0s):
                for j in js:
                    o = W2 * (j % 2)
                    nc.vector.tensor_mul(out=dst[0:64, c0s[j]:c0s[j] + N],
                                         in0=avs[j][0:HD, 0:N],
                                         in1=rec_sb[0:64, o:o + N])
                    nc.vector.tensor_mul(out=dst[64:128, c0s[j]:c0s[j] + N],
                                         in0=avs[j][0:HD, N:W2],
                                         in1=rec_sb[64:128, o + N:o + W2])

            def norm_prep(js):
                # softmax denominators for two j's: copy the PSUM rows to
                # SBUF on the scalar engine (Copy shares the Exp table -> no
                # reload; the custom-DVE approx reciprocal reads garbage
                # from PSUM), then a packed ~18-bit reciprocal_approx_fast
                # on DVE and a gpsimd partition_broadcast. Emitted in two
                # halves mid-wave so the chain finishes before the next
                # wave needs the av banks; only the multiplies are deferred.
                d2 = rec_pool.tile([1, 2 * W2], F32, tag="d2")
                for i, j in enumerate(js):
                    nc.scalar.activation(out=d2[0:1, W2 * i:W2 * (i + 1)],
                                         in_=avs[j][HD:HD + 1, :],
                                         func=AF.Copy)
                rec2 = rec_pool.tile([1, 2 * W2], F32, tag="rec2")
                nc.vector.reciprocal_approx_fast(out=rec2[0:1, :],
                                                 in_=d2[0:1, :])
                rec_sb = rec_pool.tile([128, 2 * W2], F32, tag="rec_sb2")
                nc.gpsimd.partition_broadcast(rec_sb[:], rec2[0:1, :])
                return rec_sb

            def norm_eager_j(j):
                # last pair: per-j chain so the proj tail isn't gated on a
                # whole-wave normalize (cuts ~4us off the tail)
                d1 = rec_pool.tile([1, W2], F32, tag="d1")
                nc.scalar.activation(out=d1[0:1, :],
                                     in_=avs[j][HD:HD + 1, :], func=AF.Copy)
                r1 = rec_pool.tile([1, W2], F32, tag="r1")
                nc.vector.reciprocal_approx_fast(out=r1[0:1, :], in_=d1[0:1, :])
                r1_sb = rec_pool.tile([128, W2], F32, tag="r1_sb")
                nc.gpsimd.partition_broadcast(r1_sb[:], r1[0:1, :])
                nc.vector.tensor_mul(out=dst[0:64, c0s[j]:c0s[j] + N],
                                     in0=avs[j][0:HD, 0:N],
                                     in1=r1_sb[0:64, 0:N])
                nc.vector.tensor_mul(out=dst[64:128, c0s[j]:c0s[j] + N],
                                     in0=avs[j][0:HD, N:W2],
                                     in1=r1_sb[64:128, N:W2])

            pending = [emit_scores(0), emit_scores(1)]
            # filler PE work (next pair's qk GEMMs / v GEMMs / proj) lands
            # HERE -- after this wave's first scores, so the PE chews on it
            # exactly while the scalar engine works through the exps the
            # first AVs depend on (was a ~1.5us PE stall per wave)
            filler = fillers[wave] if wave < len(fillers) else None
            if filler is not None:
                filler()
            # previous wave's first normalize half: frees av banks for this
            # wave's first AVs without stacking 8 multiplies on DVE at once
            if pending_norm:
                pending_norm.pop(0)()
            emit_av(0, pending[0])
            pending.append(emit_scores(2))
            if pending_norm:
                pending_norm.pop(0)()
            emit_av(1, pending[1])
            if eager:
                norm_eager_j(0)
                norm_eager_j(1)
            else:
                # first prep half here: its dcopies land on the scalar
                # queue BEFORE j3's exps, so the recip/broadcast chain
                # finishes inside this wave and the next wave's first AVs
                # find their banks already freeable
                r01 = norm_prep((0, 1))
                pending_norm.append(
                    lambda r=r01, f=norm_muls: f((0, 1), r))
            pending.append(emit_scores(3))
            emit_av(2, pending[2])
            emit_av(3, pending[3])
            if eager:
                norm_eager_j(2)
                norm_eager_j(3)
            else:
                r23 = norm_prep((2, 3))
                pending_norm.append(
                    lambda r=r23, f=norm_muls: f((2, 3), r))

    # ---------------- proj ----------------
    NRC = (R + 127) // 128  # 13 row chunks

    def emit_proj(rcs):
        for rc in rcs:
            r0 = 128 * rc
            nr = min(128, R - r0)
            ot = out_pool.tile([128, C], BF16, tag="out")
            for nh in range(2):
                ps = ps_pool.tile([128, 384], F32, tag="ps")
                for kc in range(KC):
                    nc.tensor.matmul(
                        out=ps[:nr, :],
                        lhsT=attn_outT[kc][:, r0:r0 + nr],
                        rhs=wp_all[:, kc, 384 * nh:384 * (nh + 1)],
                        start=(kc == 0), stop=(kc == KC - 1))
                nc.vector.tensor_add(out=ot[:nr, 384 * nh:384 * (nh + 1)],
                                     in0=ps[:nr, :],
                                     in1=bproj_bc[:nr, 384 * nh:384 * (nh + 1)])
            # one [*, 768] store per row chunk; alternate store queues so
            # descriptor issue parallelizes across the sync and gpsimd DGE
            eng = nc.sync if rc % 2 == 0 else nc.gpsimd
            eng.dma_start(out=out_d[r0:r0 + nr, :], in_=ot[:nr, :])

    # ---------------- emission schedule ----------------
    # ramp: q/k for pair 0 (kc-outer so compute starts while weight DMA
    # chunks still arrive), v for the first wave's batches
    emit_qk(0)
    emit_qk(6)
    emit_v(range(0, 4))

    def _qk(pn, ncols):
        return lambda: emit_qk_pair(pn, 6 + pn, ncols=ncols)

    # per pair p: wave-0 filler = p's own qk chunks 2,3 (needed by wave 1),
    # wave-1 filler = next pair's chunks 0,1 (needed by its wave 0).
    # pair 0's wave-0 filler is the remaining v GEMMs instead (its chunks
    # 2,3 came from the ramp).
    for p in range(NPAIR - 1):
        fa = (lambda: emit_v(range(4, 8))) if p == 0 else _qk(p, (2, 3))
        emit_attention_pair(p, fillers=(fa, _qk(p + 1, (0, 1))))

    # last pair: wave 0 (batches 0-3) with its chunks 2,3 as filler, then
    # wave 1 with the first proj row-chunks as filler, then the rest.
    # eager=True normalizes per-j so proj isn't gated on a wave-wide chain.
    emit_attention_pair(NPAIR - 1, waves=(0,), eager=True,
                        fillers=(_qk(NPAIR - 1, (2, 3)),))
    emit_attention_pair(NPAIR - 1, waves=(1,), eager=True,
                        fillers=(None, lambda: emit_proj(range(0, 6))))
    emit_proj(range(6, NRC))


_NC_CACHE = {}


def _get_nc():
    if "nc" not in _NC_CACHE:
        _NC_CACHE["nc"] = build_program()
    return _NC_CACHE["nc"]


def prep_aux(rpb_table, rel_idx):
    """Host-side prep: gather the bias from the two small aux inputs, lay it
    out per head PAIR in the kernel's transposed plane orientation
    [k_tok, q_tok*2] with zeroed CLS row/col, and exponentiate (bf16)."""
    import ml_dtypes
    bT = rpb_table[rel_idx.reshape(-1)].reshape(NP, NP, H)  # [q_idx, k_idx, h]
    bT = np.ascontiguousarray(bT.transpose(1, 0, 2))        # [k_idx, q_idx, h]
    bias0 = np.zeros((128, N, H), dtype=np.float32)
    bias0[1:128, 1:NP + 1, :] = bT[0:127]
    bias1 = np.zeros((128, N, H), dtype=np.float32)
    bias1[0:NP - 127, 1:NP + 1, :] = bT[127:NP]
    # partition-major [128, H, W2] so the device loads it as one DMA
    expb = np.zeros((128, H, W2), dtype=np.float32)
    for h in range(H):
        expb[:, h, 0:N] = np.exp(bias0[:, :, h])
        expb[:, h, N:W2] = np.exp(bias1[:, :, h])
    return expb.reshape(128, H * W2).astype(ml_dtypes.bfloat16)


def prep_weights(w_qkv, w_proj):
    """Host-side prep: transpose, fold the q scale into w_qkv, cast bf16."""
    import ml_dtypes
    wqkvT = np.array(w_qkv, dtype=np.float32).T.copy()
    wqkvT[:, 0:C] *= HD ** -0.5
    wpT = np.ascontiguousarray(np.asarray(w_proj, dtype=np.float32).T)
    return (wqkvT.astype(ml_dtypes.bfloat16), wpT.astype(ml_dtypes.bfloat16))


def make_in_maps(x, w_qkv, w_proj, b_proj, rpb_table, rel_idx):
    """Build the 8 per-core input maps (host prep: shard, transpose, bf16)."""
    import ml_dtypes
    x = np.asarray(x, dtype=np.float32)
    expb = prep_aux(
        np.asarray(rpb_table, dtype=np.float32), np.asarray(rel_idx).astype(np.int64))
    wqkvT, wpT = prep_weights(w_qkv, w_proj)
    bp = np.ascontiguousarray(np.asarray(b_proj, dtype=np.float32))
    xbf = x.astype(ml_dtypes.bfloat16)
    in_maps = []
    for c in range(NCORES):
        xT = np.ascontiguousarray(xbf[c * B:(c + 1) * B].reshape(R, C).T)
        in_maps.append({
            "xT": xT,
            "w_qkvT": wqkvT,
            "w_projT": wpT,
            "b_proj": bp,
            "expb": expb,
        })
    return in_maps


def kernel(x, w_qkv, w_proj, b_proj, rpb_table, rel_idx):
    from concourse.bass_utils import run_bass_kernel_spmd

    nc = _get_nc()
    in_maps = make_in_maps(x, w_qkv, w_proj, b_proj, rpb_table, rel_idx)
    res = run_bass_kernel_spmd(nc, in_maps, list(range(NCORES)))
    out = np.concatenate(
        [np.asarray(r["out"], dtype=np.float32).reshape(B, N, C)
         for r in res.results], axis=0)
    return out


# revision 40
# speedup vs baseline: 1.9035x; 1.0784x over previous
"""RPE (relative-position-bias) attention kernel for Trainium2, 8-core SPMD.

Full op (per reference):
  qkv = x @ w_qkv.T -> split q,k,v heads (H=12, hd=64), q *= hd**-0.5
  attn = q @ k.T ; attn[:, :, 1:, 1:] += rpb_table[rel_idx]  (per head)
  attn = softmax(attn, -1) ; out = (attn @ v) @ w_proj.T + b_proj

Sharding: data-parallel over batch. B=64 -> 8 batches per core. Weights
and bias-derived planes replicated to all cores. No collectives.

Per-core program (all matmuls bf16 operands, fp32 PSUM accumulation):
  - Inputs arrive bf16 and pre-transposed from the host: xT [768,1576],
    wqkvT [768,2304] (q columns pre-scaled by hd**-0.5), wpT [768,768].
  - qT,kT [768,1576] = w_chunk.T @ xT (transposed layout). v in natural
    layout [tokens, head, 65] with a ones column (softmax denominators
    fall out of the AV matmul for free).
  - The relative-position bias enters as exp(bias): probs = exp(s) *
    expb, expb planes host-precomputed bf16 in the transposed
    orientation [k_tok, q_tok] per head PAIR (two heads side by side,
    394 columns). exp runs on the scalar engine straight out of PSUM;
    the expb multiply runs on DVE in SBUF (bf16 fast path).
  - Heads are processed in pairs: score tiles [128,394] hold two heads.
  - Softmax normalization: denominators live in row 64 of the AV PSUM
    tile. Per half-wave (2 batches): the 2 denominator rows are copied
    PSUM->SBUF on the scalar engine (Copy shares the Exp act table, so no
    table reload; the scalar-recip variant paid a 1.3us Exp<->Recip
    reload per wave, and the custom-DVE approx reciprocal reads garbage
    from PSUM), then one packed reciprocal_approx_fast (~18-bit, 5x
    faster than the exact DVE InstReciprocal's 2.6us/row) and one gpsimd
    partition_broadcast. Only the normalize multiplies (DVE) defer to the
    next wave, flushed in halves right before the av banks are reused.
  - Engine balance: exp + qk/denominator copies on scalar; expb muls,
    norm muls, reciprocals + half the casts on DVE; broadcasts on gpsimd.
  - Emission schedule: each wave's first two score groups are emitted,
    then the "filler" PE work (next pair's qk GEMM chunks / v GEMMs /
    first proj chunks) lands while the scalar engine works through the
    exps the first AVs depend on. qk chunks 2,3 of pair p fill its own
    wave 0; chunks 0,1 of pair p+1 fill wave 1. The last pair normalizes
    per-j (eager) so the proj tail isn't gated on a wave-wide chain.
  - out = attn_outT.T @ wpT + b_proj, one [128,768] store per row chunk.
"""
import sys

sys.path.insert(0, '/opt/trn_rl_repo')

from contextlib import ExitStack

import numpy as np

import concourse.bass as bass
import concourse.bacc as bacc
import concourse.tile as tile
from concourse import mybir

# ---- problem dims (hardcoded per contract) ----
NCORES = 8
B_FULL = 64
B = B_FULL // NCORES     # 8 batches per core
N = 197                  # tokens (196 patches + CLS)
NP = 196
C = 768
H = 12
HD = 64
R = B * N                # 1576 rows per core
NPAIR = H // 2           # 6 head pairs
W2 = 2 * N               # 394 columns for a head pair

F32 = mybir.dt.float32
BF16 = mybir.dt.bfloat16
AF = mybir.ActivationFunctionType

import os
# CoreSim rejects reads of uninitialized PSUM; the exp deliberately reads a
# dead corner of the score tile (rows 69:128 of the chunk-1 columns, never
# consumed downstream). Sim runs memset it; hardware runs skip the cost.
SIM_SAFE = int(os.environ.get("KERNEL_SIM_SAFE", "0"))


def build_program():
    nc = bacc.Bacc("TRN2", target_bir_lowering=False, debug=False)

    x_d = nc.declare_dram_parameter("xT", [C, R], BF16, isOutput=False)
    wqkv_d = nc.declare_dram_parameter("w_qkvT", [C, 3 * C], BF16, isOutput=False)
    wp_d = nc.declare_dram_parameter("w_projT", [C, C], BF16, isOutput=False)
    bp_d = nc.declare_dram_parameter("b_proj", [C], F32, isOutput=False)
    # exp(bias) planes per head, transposed chunk-paired orientation:
    # expb [k_part 0:128, head, q 0:197 (k chunk 0) ++ q 0:197 (k chunk 1)]
    # (chunk 1 rows beyond k=196 are 1.0 and multiply unused garbage);
    # partition-major host layout so it loads as ONE contiguous DMA
    expb_d = nc.declare_dram_parameter("expb", [128, H * W2], BF16,
                                       isOutput=False)
    out_d = nc.declare_dram_parameter("out", [R, C], BF16, isOutput=True)

    with tile.TileContext(nc) as tc:
        with ExitStack() as ctx:
            _emit(ctx, tc, nc, x_d, wqkv_d, wp_d, bp_d, expb_d, out_d)
    nc.compile()
    return nc


def _emit(ctx, tc, nc, x_d, wqkv_d, wp_d, bp_d, expb_d, out_d):
    singles = ctx.enter_context(tc.tile_pool(name="singles", bufs=1))
    ps_pool = ctx.enter_context(tc.tile_pool(name="ps", bufs=4, space="PSUM"))
    av_ps = ctx.enter_context(tc.tile_pool(name="av_ps", bufs=4, space="PSUM"))
    probs_pool = ctx.enter_context(tc.tile_pool(name="probs", bufs=12))
    rec_pool = ctx.enter_context(tc.tile_pool(name="rec", bufs=2))
    out_pool = ctx.enter_context(tc.tile_pool(name="outp", bufs=4))

    KC = C // 128  # 6 contraction chunks

    # ---------------- load operands (already bf16 + transposed) ----------
    xT = []     # 6 x [128, R] bf16
    wqkvT = []  # 6 x [128, 1536] bf16 (q,k columns; v loads separately)
    for kc in range(KC):
        t = singles.tile([128, R], BF16, tag=f"xT{kc}", name=f"xT{kc}")
        # split the x chunks across the gpsimd and scalar DGE queues so
        # the later contraction chunks aren't serialized behind the first
        # ones' descriptor issue (the ramp's qk GEMMs consume them at
        # ~1.6us/chunk, about the single-queue delivery rate)
        eng = nc.gpsimd if kc % 2 == 0 else nc.scalar
        eng.dma_start(out=t[:], in_=x_d[128 * kc:128 * (kc + 1), :])
        xT.append(t)
        w = singles.tile([128, 2 * C], BF16, tag=f"wqkvT{kc}", name=f"wqkvT{kc}")
        nc.sync.dma_start(out=w[:],
                          in_=wqkv_d[128 * kc:128 * (kc + 1), 0:2 * C])
        wqkvT.append(w)
    # remaining inputs issue from the (idle at startup) scalar DGE queue so the xT/wqkv chunk DMAs aren't stuck behind them.
    # v columns of w_qkv: one strided DMA filling all six chunk tiles'
    # v-column ranges (6 descriptors/partition instead of 6 DMA issues)
    vcols = singles.tile([128, KC, C], BF16, tag="wqkv_v", name="wqkv_v")
    nc.scalar.dma_start(
        out=vcols[:],
        in_=bass.AP(tensor=wqkv_d, offset=2 * C,
                    ap=[[3 * C, 128], [128 * 3 * C, KC], [1, C]]))
    # wp: one strided DMA for all 6 contraction chunks
    wp_all = singles.tile([128, KC, C], BF16, tag="wp_all", name="wp_all")
    nc.scalar.dma_start(
        out=wp_all[:],
        in_=bass.AP(tensor=wp_d, offset=0,
                    ap=[[C, 128], [128 * C, KC], [1, C]]))

    bproj_bc = singles.tile([128, C], F32, tag="bproj")
    nc.scalar.dma_start(out=bproj_bc[:],
                        in_=bass.AP(tensor=bp_d, offset=0, ap=[[0, 128], [1, C]]))

    expb_all = singles.tile([128, H, W2], BF16, tag="expb", name="expb")
    nc.scalar.dma_start(out=expb_all[:], in_=expb_d[:, :])
    expb = [expb_all[:, h, :] for h in range(H)]  # [128, 394] per head

    # ---------------- QKV ----------------
    NCHUNK = 4
    NW = R // NCHUNK  # 394 columns per psum tile

    # qk_sb[ft][ncol]: SEPARATE [128, 394] tiles per column chunk (2
    # batches each). Dependency tracking is tile-granular for matmul
    # operands, so per-chunk tiles let a wave's scores wait on exactly the
    # one copy they need instead of all four.
    qk_sb = [[None] * NCHUNK for _ in range(12)]

    def emit_qk(ft):
        # kc-outer, all four column tiles held across the contraction: the
        # first chunks compute while later weight DMA chunks still arrive
        # (only matters for the ramp pair ft 0/6)
        tiles = [ps_pool.tile([128, NW], F32, tag="ps", name=f"qkps{ncol}")
                 for ncol in range(NCHUNK)]
        for kc in range(KC):
            for ncol in range(NCHUNK):
                nc.tensor.matmul(
                    out=tiles[ncol][:],
                    lhsT=wqkvT[kc][:, 128 * ft:128 * (ft + 1)],
                    rhs=xT[kc][:, NW * ncol:NW * (ncol + 1)],
                    start=(kc == 0), stop=(kc == KC - 1))
        for ncol in range(NCHUNK):
            tg = (ft % 6) % 3 + (3 if ft >= 6 else 0)
            dst = singles.tile([128, NW], BF16, tag=f"qk{tg}_{ncol}",
                               name=f"qk{ft}_{ncol}")
            qk_sb[ft][ncol] = dst
            nc.vector.tensor_copy(out=dst[:], in_=tiles[ncol][:])

    def qk_units(fta, ftb, ncols):
        # q/k GEMM "filler units": one closure per (ncol, ft) -- a 6-chunk
        # GEMM plus its PSUM->SBUF copy. The wave scheduler sprinkles these
        # at its dependency edges. Copies alternate scalar-Copy (shares the
        # Exp act table, no reload) and DVE.
        def unit(ncol, i, ft):
            ps = ps_pool.tile([128, NW], F32, tag="ps", name="qkps")
            for kc in range(KC):
                nc.tensor.matmul(
                    out=ps[:],
                    lhsT=wqkvT[kc][:, 128 * ft:128 * (ft + 1)],
                    rhs=xT[kc][:, NW * ncol:NW * (ncol + 1)],
                    start=(kc == 0), stop=(kc == KC - 1))
            tg = (ft % 6) % 3 + (3 if ft >= 6 else 0)
            dst = singles.tile([128, NW], BF16, tag=f"qk{tg}_{ncol}",
                               name=f"qk{ft}_{ncol}")
            qk_sb[ft][ncol] = dst
            if (ncol + i) % 2 == 0:
                nc.scalar.activation(out=dst[:], in_=ps[:], func=AF.Copy)
            else:
                nc.vector.tensor_copy(out=dst[:], in_=ps[:])
        return [lambda n=ncol, i=i, f=ft: unit(n, i, f)
                for ncol in ncols for i, ft in enumerate((fta, ftb))]

    # v_aug[b][c]: [128, 12, 65] bf16 (col 64 = ones)
    v_aug = [[None, None] for _ in range(B)]

    def _v_unit(b, cchunk):
        r0, nr = (N * b, 128) if cchunk == 0 else (N * b + 128, N - 128)
        dst = singles.tile([128, H, HD + 1], BF16, tag=f"v{b}_{cchunk}",
                           name=f"v{b}_{cchunk}")
        v_aug[b][cchunk] = dst
        nc.vector.memset(dst[:, :, HD:HD + 1], 1.0)
        for nh in range(2):
            ps = ps_pool.tile([128, 384], F32, tag="ps")
            for kc in range(KC):
                nc.tensor.matmul(
                    out=ps[:nr, :],
                    lhsT=xT[kc][:, r0:r0 + nr],
                    rhs=vcols[:, kc, 384 * nh:384 * (nh + 1)],
                    start=(kc == 0), stop=(kc == KC - 1))
            # PSUM->SBUF casts split DVE / scalar-Copy (gpsimd cannot
            # read PSUM); both engines are idle at this schedule point
            if (b + cchunk + nh) % 2 == 0:
                nc.vector.tensor_copy(
                    out=dst[:nr, 6 * nh:6 * (nh + 1), 0:HD],
                    in_=ps[:nr, :].rearrange("p (h d) -> p h d", h=6))
            else:
                nc.scalar.activation(
                    out=dst[:nr, 6 * nh:6 * (nh + 1), 0:HD],
                    in_=ps[:nr, :].rearrange("p (h d) -> p h d", h=6),
                    func=AF.Copy)

    def v_units(batches):
        return [lambda b=b, c=c: _v_unit(b, c)
                for b in batches for c in range(2)]

    def emit_v(batches):
        for u in v_units(batches):
            u()

    # deferred normalize-multiply closures (two half-flushes per wave)
    pending_norm = []

    def flush_norm():
        while pending_norm:
            pending_norm.pop(0)()

    # attn output, transposed: 6 tiles [128, R] bf16 (pair p = heads 2p,2p+1)
    attn_outT = []
    for p in range(NPAIR):
        attn_outT.append(singles.tile([128, R], BF16, tag=f"aoT{p}",
                                      name=f"aoT{p}"))

    def emit_attention_pair(p, waves=(0, 1), fillers=(None, None),
                            eager=False):
        N1 = N - 128  # 69
        dst = attn_outT[p]
        for wave in waves:
            avs = []
            c0s = []

            def emit_scores(j):
                b = 4 * wave + j
                c0s.append(N * b)
                qt = qk_sb[p][b // 2]
                kt = qk_sb[6 + p][b // 2]
                o = N * (b % 2)
                qh = [qt[0:64, o:o + N], qt[64:128, o:o + N]]
                kh = [kt[0:64, o:o + N], kt[64:128, o:o + N]]
                phs = []
                for hh in range(2):
                    sth = ps_pool.tile([128, W2], F32, tag="ps",
                                       name=f"sth{hh}")
                    if SIM_SAFE:
                        nc.vector.memset(sth[64:128, N:W2], 0.0)
                    nc.tensor.matmul(out=sth[:, 0:N],
                                     lhsT=kh[hh][:, 0:128], rhs=qh[hh],
                                     start=True, stop=True)
                    nc.tensor.matmul(out=sth[0:N1, N:W2],
                                     lhsT=kh[hh][:, 128:N], rhs=qh[hh],
                                     start=True, stop=True)
                    ph = probs_pool.tile([128, W2], BF16, tag="probs")
                    nc.scalar.activation(out=ph[:], in_=sth[:], func=AF.Exp)
                    # expb multiply: all-SBUF bf16 fast DVE path, kept on
                    # DVE only -- on gpsimd it sat behind the broadcast in
                    # the in-order queue and stalled the AV matmuls
                    nc.vector.tensor_mul(out=ph[:], in0=ph[:],
                                         in1=expb[2 * p + hh])
                    phs.append(ph)
                return phs

            def emit_av(j, phs):
                b = 4 * wave + j
                av = av_ps.tile([HD + 1, W2], F32, tag="av")
                avs.append(av)
                for hh in range(2):
                    h = 2 * p + hh
                    nc.tensor.matmul(out=av[:, N * hh:N * hh + N],
                                     lhsT=v_aug[b][0][:, h, :],
                                     rhs=phs[hh][:, 0:N],
                                     start=True, stop=False)
                    nc.tensor.matmul(out=av[:, N * hh:N * hh + N],
                                     lhsT=v_aug[b][1][0:N1, h, :],
                                     rhs=phs[hh][0:N1, N:W2],
                                     start=False, stop=True)

            def norm_muls(js, rec_sb, avs=avs, c0s=c0s):
                for j in js:
                    o = W2 * (j % 2)
                    nc.vector.tensor_mul(out=dst[0:64, c0s[j]:c0s[j] + N],
                                         in0=avs[j][0:HD, 0:N],
                                         in1=rec_sb[0:64, o:o + N])
                    nc.vector.tensor_mul(out=dst[64:128, c0s[j]:c0s[j] + N],
                                         in0=avs[j][0:HD, N:W2],
                                         in1=rec_sb[64:128, o + N:o + W2])

            def norm_prep(js):
                # softmax denominators for two j's: copy the PSUM rows to
                # SBUF on the scalar engine (Copy shares the Exp table -> no
                # reload; the custom-DVE approx reciprocal reads garbage
                # from PSUM), then a packed ~18-bit reciprocal_approx_fast
                # on DVE and a gpsimd partition_broadcast. Emitted in two
                # halves mid-wave so the chain finishes before the next
                # wave needs the av banks; only the multiplies are deferred.
                d2 = rec_pool.tile([1, 2 * W2], F32, tag="d2")
                for i, j in enumerate(js):
                    nc.scalar.activation(out=d2[0:1, W2 * i:W2 * (i + 1)],
                                         in_=avs[j][HD:HD + 1, :],
                                         func=AF.Copy)
                rec2 = rec_pool.tile([1, 2 * W2], F32, tag="rec2")
                nc.vector.reciprocal_approx_fast(out=rec2[0:1, :],
                                                 in_=d2[0:1, :])
                rec_sb = rec_pool.tile([128, 2 * W2], F32, tag="rec_sb2")
                nc.gpsimd.partition_broadcast(rec_sb[:], rec2[0:1, :])
                return rec_sb

            def norm_eager_j(j):
                # last pair: per-j chain so the proj tail isn't gated on a
                # whole-wave normalize (cuts ~4us off the tail)
                d1 = rec_pool.tile([1, W2], F32, tag="d1")
                nc.scalar.activation(out=d1[0:1, :],
                                     in_=avs[j][HD:HD + 1, :], func=AF.Copy)
                r1 = rec_pool.tile([1, W2], F32, tag="r1")
                nc.vector.reciprocal_approx_fast(out=r1[0:1, :], in_=d1[0:1, :])
                r1_sb = rec_pool.tile([128, W2], F32, tag="r1_sb")
                nc.gpsimd.partition_broadcast(r1_sb[:], r1[0:1, :])
                nc.vector.tensor_mul(out=dst[0:64, c0s[j]:c0s[j] + N],
                                     in0=avs[j][0:HD, 0:N],
                                     in1=r1_sb[0:64, 0:N])
                nc.vector.tensor_mul(out=dst[64:128, c0s[j]:c0s[j] + N],
                                     in0=avs[j][0:HD, N:W2],
                                     in1=r1_sb[64:128, N:W2])

            pending = [emit_scores(0), emit_scores(1)]
            # filler PE work (next pair's qk GEMMs / v GEMMs / proj) is a
            # list of units sprinkled at the wave's dependency edges, so
            # the PE always has independent work while the scalar/DVE
            # engines run the exp->mul chains the next AV depends on
            units = list(fillers[wave]) if (wave < len(fillers)
                                            and fillers[wave]) else []

            def sprinkle(n):
                for _ in range(min(n, len(units))):
                    units.pop(0)()
            sprinkle(2)
            # previous wave's first normalize half: frees av banks for this
            # wave's first AVs without stacking 8 multiplies on DVE at once
            if pending_norm:
                pending_norm.pop(0)()
            emit_av(0, pending[0])
            pending.append(emit_scores(2))
            sprinkle(1)
            if pending_norm:
                pending_norm.pop(0)()
            emit_av(1, pending[1])
            if eager:
                norm_eager_j(0)
                norm_eager_j(1)
            else:
                # first prep half here: its dcopies land on the scalar
                # queue BEFORE j3's exps, so the recip/broadcast chain
                # finishes inside this wave and the next wave's first AVs
                # find their banks already freeable
                r01 = norm_prep((0, 1))
                pending_norm.append(
                    lambda r=r01, f=norm_muls: f((0, 1), r))
            pending.append(emit_scores(3))
            sprinkle(1)
            emit_av(2, pending[2])
            emit_av(3, pending[3])
            if eager:
                norm_eager_j(2)
                norm_eager_j(3)
            else:
                r23 = norm_prep((2, 3))
                pending_norm.append(
                    lambda r=r23, f=norm_muls: f((2, 3), r))
            sprinkle(len(units))

    # ---------------- proj ----------------
    NRC = (R + 127) // 128  # 13 row chunks

    def emit_proj(rcs):
        for rc in rcs:
            r0 = 128 * rc
            nr = min(128, R - r0)
            ot = out_pool.tile([128, C], BF16, tag="out")
            for nh in range(2):
                ps = ps_pool.tile([128, 384], F32, tag="ps")
                for kc in range(KC):
                    nc.tensor.matmul(
                        out=ps[:nr, :],
                        lhsT=attn_outT[kc][:, r0:r0 + nr],
                        rhs=wp_all[:, kc, 384 * nh:384 * (nh + 1)],
                        start=(kc == 0), stop=(kc == KC - 1))
                nc.vector.tensor_add(out=ot[:nr, 384 * nh:384 * (nh + 1)],
                                     in0=ps[:nr, :],
                                     in1=bproj_bc[:nr, 384 * nh:384 * (nh + 1)])
            # one [*, 768] store per row chunk; alternate store queues so
            # descriptor issue parallelizes across the sync and gpsimd DGE
            eng = nc.sync if rc % 2 == 0 else nc.gpsimd
            eng.dma_start(out=out_d[r0:r0 + nr, :], in_=ot[:nr, :])

    # ---------------- emission schedule ----------------
    # ramp: q/k for pair 0 (kc-outer so compute starts while weight DMA
    # chunks still arrive), v for the first wave's batches
    emit_qk(0)
    emit_qk(6)
    emit_v(range(0, 4))

    # per pair p: wave-0 filler = p's own qk chunks 2,3 (needed by wave 1),
    # wave-1 filler = next pair's chunks 0,1 (needed by its wave 0).
    # pair 0's wave-0 filler is the remaining v GEMMs instead (its chunks
    # 2,3 came from the ramp).
    for p in range(NPAIR - 1):
        fa = v_units(range(4, 8)) if p == 0 else qk_units(p, 6 + p, (2, 3))
        emit_attention_pair(p, fillers=(fa, qk_units(p + 1, 7 + p, (0, 1))))

    # last pair: wave 0 (batches 0-3) with its chunks 2,3 as filler, then
    # wave 1 with the first proj row-chunks as filler, then the rest.
    # eager=True normalizes per-j so proj isn't gated on a wave-wide chain.
    emit_attention_pair(NPAIR - 1, waves=(0,), eager=True,
                        fillers=(qk_units(NPAIR - 1, 2 * NPAIR - 1, (2, 3)),))
    emit_attention_pair(NPAIR - 1, waves=(1,), eager=True,
                        fillers=(None, [lambda rc=rc: emit_proj([rc])
                                        for rc in range(0, 6)]))
    emit_proj(range(6, NRC))


_NC_CACHE = {}


def _get_nc():
    if "nc" not in _NC_CACHE:
        _NC_CACHE["nc"] = build_program()
    return _NC_CACHE["nc"]


def prep_aux(rpb_table, rel_idx):
    """Host-side prep: gather the bias from the two small aux inputs, lay it
    out per head PAIR in the kernel's transposed plane orientation
    [k_tok, q_tok*2] with zeroed CLS row/col, and exponentiate (bf16)."""
    import ml_dtypes
    bT = rpb_table[rel_idx.reshape(-1)].reshape(NP, NP, H)  # [q_idx, k_idx, h]
    bT = np.ascontiguousarray(bT.transpose(1, 0, 2))        # [k_idx, q_idx, h]
    bias0 = np.zeros((128, N, H), dtype=np.float32)
    bias0[1:128, 1:NP + 1, :] = bT[0:127]
    bias1 = np.zeros((128, N, H), dtype=np.float32)
    bias1[0:NP - 127, 1:NP + 1, :] = bT[127:NP]
    # partition-major [128, H, W2] so the device loads it as one DMA
    expb = np.zeros((128, H, W2), dtype=np.float32)
    for h in range(H):
        expb[:, h, 0:N] = np.exp(bias0[:, :, h])
        expb[:, h, N:W2] = np.exp(bias1[:, :, h])
    return expb.reshape(128, H * W2).astype(ml_dtypes.bfloat16)


def prep_weights(w_qkv, w_proj):
    """Host-side prep: transpose, fold the q scale into w_qkv, cast bf16."""
    import ml_dtypes
    wqkvT = np.array(w_qkv, dtype=np.float32).T.copy()
    wqkvT[:, 0:C] *= HD ** -0.5
    wpT = np.ascontiguousarray(np.asarray(w_proj, dtype=np.float32).T)
    return (wqkvT.astype(ml_dtypes.bfloat16), wpT.astype(ml_dtypes.bfloat16))


def make_in_maps(x, w_qkv, w_proj, b_proj, rpb_table, rel_idx):
    """Build the 8 per-core input maps (host prep: shard, transpose, bf16)."""
    import ml_dtypes
    x = np.asarray(x, dtype=np.float32)
    expb = prep_aux(
        np.asarray(rpb_table, dtype=np.float32), np.asarray(rel_idx).astype(np.int64))
    wqkvT, wpT = prep_weights(w_qkv, w_proj)
    bp = np.ascontiguousarray(np.asarray(b_proj, dtype=np.float32))
    xbf = x.astype(ml_dtypes.bfloat16)
    in_maps = []
    for c in range(NCORES):
        xT = np.ascontiguousarray(xbf[c * B:(c + 1) * B].reshape(R, C).T)
        in_maps.append({
            "xT": xT,
            "w_qkvT": wqkvT,
            "w_projT": wpT,
            "b_proj": bp,
            "expb": expb,
        })
    return in_maps


def kernel(x, w_qkv, w_proj, b_proj, rpb_table, rel_idx):
    from concourse.bass_utils import run_bass_kernel_spmd

    nc = _get_nc()
    in_maps = make_in_maps(x, w_qkv, w_proj, b_proj, rpb_table, rel_idx)
    res = run_bass_kernel_spmd(nc, in_maps, list(range(NCORES)))
    out = np.concatenate(
        [np.asarray(r["out"], dtype=np.float32).reshape(B, N, C)
         for r in res.results], axis=0)
    return out
